# revision 58
# baseline (speedup 1.0000x reference)
"""CRF negative-log-likelihood kernel for Trainium2 (8 NeuronCores).

Math: reference computes  partition - gold  where
  partition = sum_b logsumexp_c(alpha[511])  via the forward algorithm
  gold      = sum emissions[b,s,tags] * m + sum T[tags[s],tags[s+1]] * m[:,1:]

Device strategy (data-parallel over batch, 32 rows per core):
  * Linear domain: alpha_t = E_t o (A^T alpha_{t-1}) with A = exp(T),
    E_t = exp(e_t - MU).  One [128,128]x[128,W] matmul (PE) plus one
    elementwise multiply (DVE) per step.
  * K=23 overlapping forward chains cut the serial depth from 511 steps
    to L-1=27.  Chain j starts at t = j*DELTA from the raw emission
    vector E_{j*DELTA} and runs L=28 steps; its first O=5 steps are
    warm-up inside chain j-1's range.  Products of >=5 random positive
    matrices are numerically rank-1 (Perron-Frobenius contraction), so
    the chains glue exactly through two column-sum scalars per junction:
      logZ_b = log n[K-1] + sum_j (log n[j-1] - log gamma[j]) + MU*S
    where gamma[j] = colsum of chain j's state after its warm-up step O
    and n[j] = colsum at its final step (both measure t = j*DELTA + O
    resp. j*DELTA + L-1; the grid aligns junctions exactly).  Host takes
    the logs in f64.  Validated: junction error ~1e-16, total loss
    rel err ~3e-5 (bf16/fp8 rounding dominated).
  * No renormalisation: the exp bias -MU keeps per-step growth ~1, and
    a 28-step chain drifts far less than the f32/bf16 exponent range.
  * The scan runs as G=2 independent chain-groups (12+11 chains wide)
    round-robined so the DVE (the bottleneck engine: 125ns PSUM-access
    init + 1.04ns/col) stays saturated while semaphore round-trips hide.
  * Emissions arrive as fp8-e4m3 (halves DMA; validated noise ~1e-4) in
    a step-major block layout so DMA+exp stream strictly ahead of
    consumption and every scan-step read is one contiguous slice.
  * Gold emit: sum(raw o onehot(tags)) via PE: 128 PSUM-accumulated
    fp8 matmuls H_c^T R_c (diag trick), injected into scan-idle PE
    slots; diag extracted with an identity multiply + free-axis reduce.
  * Gold trans: host-built pair-count matrix CNT (index-only prep),
    mul+reduce against T on Pool/DVE.
  * Startup/tail: activation-table load and PE p-state ramp pre-warmed
    under the DMA shadow; weights on the parallel Pool DMA queue; gold
    reduction finishes on Pool so its DMA overlaps the csf output path.
Outputs per core: two colsum rows + gold column; host sums in float64.
"""

import sys

for _p in ("/opt/trn_rl_repo",):
    if _p not in sys.path:
        sys.path.insert(0, _p)

import os as _os
import numpy as np
import ml_dtypes
from contextlib import ExitStack

from concourse import bass, tile, mybir, bacc
from concourse.bass_utils import run_bass_kernel_spmd

NCORES = 8
B, S, C = 256, 512, 128
BC = B // NCORES          # batch rows per core
K = 23                    # chains
O = 5                     # warm-up steps per chain
DE = 22                   # chain start stride (DELTA)
L = DE + O + 1            # steps per chain (incl. init step 0)
MU = 5.85                 # exp prescale; host adds MU*S back per batch row
W = K * BC                # 736: full state width
G0W = 12 * BC             # group 0: chains 0..11  (384 cols)
G1W = 11 * BC             # group 1: chains 12..22 (352 cols)
NCOL = S * BC             # 16384 stored emission columns per core
assert K * DE == S - 1 - O and (K - 1) * DE + L - 1 == S - 1

# stored block order = consumption order: small block BLK(k) (chain 0's
# 32-col tile for t=k) directly before big block BLK(k+DE) for k=0..O,
# then big BLK(O+1..DE-1).  BLK(k>O) holds slot j = chain j's tile for
# t = j*DE + k (K*32 cols).  Small-before-big makes every warm-up read
# [chain0 | chains 1..11] one contiguous 384-col slice.
_ORDER = []
for _k in range(O + 1):
    _ORDER += [_k, DE + _k]
_ORDER += list(range(O + 1, DE))
OFF = {}
_pos = 0
for _k in _ORDER:
    OFF[_k] = _pos
    _pos += W if _k > O else BC
assert _pos == NCOL

# exp chunks: (offset, size) pairs in stored order.  Chunk c<=5 feeds
# scan step c (and steps 22..27); bulk chunk 6+i (2 blocks) feeds steps
# 6+2i and 7+2i.
CHUNKS = []
for _i in range(O + 1):                       # 6 chunks of 768
    CHUNKS.append((_i * (W + BC), W + BC))
_base = (O + 1) * (W + BC)
for _i in range(8):                           # 8 chunks of 2*736
    CHUNKS.append((_base + _i * 2 * W, 2 * W))
assert CHUNKS[-1][0] + CHUNKS[-1][1] == NCOL

# DMA batches (HWDGE costs 625ns per dma_start; batch the tail, but keep
# the early chunks separate so each lands — and unblocks its exp — ASAP).
# Chains j>0 warm-start from ones, so round 0 needs only the 32-col s0
# slice; block DE+0 (chunk 0's remainder) isn't read again until round
# DE, letting its DMA+exp drop out of the startup chain entirely.
_D1 = 3 * (W + BC)
DMA_BATCHES = [CHUNKS[1], (0, BC), CHUNKS[2],
               (_D1, _base - _D1), (_base, 2 * W),
               (_base + 2 * W, NCOL - _base - 2 * W), (BC, W)]

F32 = mybir.dt.float32
BF16 = mybir.dt.bfloat16
FP8 = mybir.dt.float8e4
AF = mybir.ActivationFunctionType
OP = mybir.AluOpType

_EN_GOLD = _os.environ.get("CRF_GOLD", "1") == "1"
_EN_SCAN = _os.environ.get("CRF_SCAN", "1") == "1"

_NC_CACHE = None


def _build_nc():
    nc = bacc.Bacc("TRN2", target_bir_lowering=False, debug=False)

    et_in = nc.dram_tensor("et", [C, NCOL], FP8, kind="ExternalInput").ap()
    hemit_in = nc.dram_tensor("hemit", [C, NCOL], FP8,
                              kind="ExternalInput").ap()
    afwd = nc.dram_tensor("afwd", [C, C], BF16, kind="ExternalInput").ap()
    cnt_in = nc.dram_tensor("cnt", [C, C], F32, kind="ExternalInput").ap()
    tsb_in = nc.dram_tensor("tsb", [C, C], F32, kind="ExternalInput").ap()
    id_in = nc.dram_tensor("ident", [C, C], BF16, kind="ExternalInput").ap()
    cso_out = nc.dram_tensor("cso", [1, W], F32, kind="ExternalOutput").ap()
    csf_out = nc.dram_tensor("csf", [1, W], F32, kind="ExternalOutput").ap()
    gold = nc.dram_tensor("gold", [C, 1], F32, kind="ExternalOutput").ap()

    with tile.TileContext(nc) as tc, ExitStack() as ctx:
        sb = ctx.enter_context(tc.tile_pool(name="sb", bufs=1))
        wk = ctx.enter_context(tc.tile_pool(name="wk", bufs=4))
        ps = ctx.enter_context(tc.tile_pool(name="ps", bufs=2, space="PSUM"))

        # ---- persistent tiles -------------------------------------------
        bias = sb.tile([C, 1], F32, name="bias")
        nc.vector.memset(bias[:], -MU)
        ones_col = sb.tile([C, 1], BF16, name="ones_col")
        nc.vector.memset(ones_col[:], 1.0)
        # dummy exp: pulls the 1283ns activation-table load into the DMA
        # shadow at t=0
        warm = sb.tile([C, 1], BF16, name="warm")
        nc.scalar.activation(warm[:], bias[:], AF.Exp, bias=bias[:])
        # dummy matmul: starts the PE p-state ramp clock (3us to full
        # speed) during the DMA shadow so round 0 runs at full clock
        pwarm = ps.tile([1, 1], F32, tag="pw", bufs=1, name="pwarm")
        nc.tensor.matmul(pwarm[:], ones_col[:], ones_col[:],
                         start=True, stop=True)

        raw = sb.tile([C, NCOL], FP8, name="raw")
        E = sb.tile([C, NCOL], BF16, name="E")
        hem = sb.tile([C, NCOL], FP8, name="hem")
        wf = sb.tile([C, C], BF16, name="wf")
        cso_sb = sb.tile([1, W], F32, name="cso_sb")
        csf_sb = sb.tile([1, W], F32, name="csf_sb")

        # ---- input DMA: et batches in consumption order (weights after
        # the early batches: not needed until the first matmul), then
        # hemit (needed mid-scan for gold), then the small gold inputs ----
        # weights ride the otherwise-idle Pool SWDGE queue, in parallel
        # with the emission stream on the SP queue
        nc.gpsimd.dma_start(wf[:], afwd[:])
        for o, n in DMA_BATCHES:
            nc.sync.dma_start(raw[:, o:o + n], et_in[:, o:o + n])
        hq = NCOL // 2
        for i in range(2):
            nc.sync.dma_start(hem[:, i * hq:(i + 1) * hq],
                              hemit_in[:, i * hq:(i + 1) * hq])
        cnt_sb = sb.tile([C, C], F32, name="cnt_sb")
        tsb = sb.tile([C, C], F32, name="tsb_t")
        ident = sb.tile([C, C], BF16, name="ident")
        nc.sync.dma_start(cnt_sb[:], cnt_in[:])
        nc.sync.dma_start(tsb[:], tsb_in[:])
        nc.sync.dma_start(ident[:], id_in[:])

        def exp_chunk(c, split=0):
            o, n = CHUNKS[c]
            if split:
                nc.scalar.activation(E[:, o:o + split], raw[:, o:o + split],
                                     AF.Exp, bias=bias[:])
                nc.scalar.activation(E[:, o + split:o + n],
                                     raw[:, o + split:o + n],
                                     AF.Exp, bias=bias[:])
            else:
                nc.scalar.activation(E[:, o:o + n], raw[:, o:o + n], AF.Exp,
                                     bias=bias[:])

        # E source slice for (step, group).  Small-before-big block order
        # makes group 0's warm-up read [chain0 | big-block slots 0..10]
        # contiguous, so both groups always read one slice starting at
        # OFF[kk] (the small block for kk<=O, the big block otherwise).
        def e_rng(kk, grp):
            if grp == 0:
                return OFF[kk], G0W
            return OFF[kk] + G0W, G1W

        # gold state
        if _EN_GOLD:
            gold_ps = ps.tile([C, C], F32, tag="gps", bufs=1, name="gold_ps")
            NGC = NCOL // C                            # 128 matmul chunks
            gpos = [0]
            ttr = sb.tile([C, C], F32, name="ttr")

            def gold_trans():
                # cnt o T multiply on Pool (idle during the scan)
                nc.gpsimd.tensor_tensor(ttr[:], cnt_sb[:], tsb[:], op=OP.mult)

            def gold_mm(nmm, anchor=None):
                from concourse.tile_rust import add_dep_helper
                for i in range(nmm):
                    m = gpos[0]
                    if m >= NGC:
                        return
                    gpos[0] += 1
                    gi = nc.tensor.matmul(
                        gold_ps[:], hem[:, m * C:(m + 1) * C],
                        raw[:, m * C:(m + 1) * C],
                        start=(m == 0), stop=(m == NGC - 1))
                    if i == 0 and anchor is not None:
                        # pin the batch into this round's PE idle window --
                        # Tile otherwise hoists it between the scan matmuls
                        add_dep_helper(gi.ins, anchor.ins,
                                       reason="gold batch after scan matmul")

            def gold_finish(anchor=None):
                from concourse.tile_rust import add_dep_helper
                gacc = sb.tile([C, 1], F32, name="gacc")
                tp = sb.tile([C, 1], F32, name="tp")
                trash = sb.tile([C, C], BF16, name="trash")
                nc.vector.tensor_tensor(trash[:], gold_ps[:], ident[:],
                                        op=OP.mult)
                nc.vector.reduce_sum(gacc[:], trash[:],
                                     axis=mybir.AxisListType.X)
                rtp = nc.vector.reduce_sum(tp[:], ttr[:],
                                           axis=mybir.AxisListType.X)
                if anchor is not None:
                    # keep this off the saturated DVE until the scan is done
                    # (its inputs are ready early and Tile hoists it)
                    add_dep_helper(rtp.ins, anchor.ins,
                                   reason="gold reduce after last scan mult")
                # final add + DMA on the idle Pool engine/queue so the gold
                # path never queues behind the csf copies on DVE
                nc.gpsimd.tensor_add(gacc[:], gacc[:], tp[:])
                nc.gpsimd.dma_start(gold[:], gacc[:])
        else:
            def gold_trans():
                pass

            def gold_mm(nmm, anchor=None):
                pass

            def gold_finish(anchor=None):
                zg = sb.tile([C, 1], F32, name="zg")
                nc.vector.memset(zg[:], 0.0)
                nc.sync.dma_start(gold[:], zg[:])

        if not _EN_SCAN:
            zr = sb.tile([1, W], F32, name="zr")
            nc.vector.memset(zr[:], 1.0)
            nc.sync.dma_start(cso_out[:], zr[:])
            nc.sync.dma_start(csf_out[:], zr[:])
            for c in range(len(CHUNKS)):
                exp_chunk(c)
            gold_trans()
            gold_mm(NCOL // C if _EN_GOLD else 0)
            gold_finish()
            nc.compile()
            return nc

        # exp the slices the first scan step needs, smallest-first so each
        # consumer (init matmuls, then the two step-1 mults) starts ASAP
        o1, n1 = CHUNKS[1]
        nc.scalar.activation(E[:, 0:BC], raw[:, 0:BC], AF.Exp, bias=bias[:])
        nc.scalar.activation(E[:, o1:o1 + G0W], raw[:, o1:o1 + G0W],
                             AF.Exp, bias=bias[:])
        nc.scalar.activation(E[:, o1 + G0W:o1 + n1], raw[:, o1 + G0W:o1 + n1],
                             AF.Exp, bias=bias[:])
        ones_mv = sb.tile([C, G1W], BF16, name="ones_mv")
        nc.vector.memset(ones_mv[:], 1.0)

        def extract_mm(state0, state1):
            # colsums via ones-matmul into PSUM (held there until copied)
            c0 = ps.tile([1, G0W], F32, tag="cs0", bufs=1, name="c0")
            c1 = ps.tile([1, G1W], F32, tag="cs1", bufs=1, name="c1")
            nc.tensor.matmul(c0[:], ones_col[:], state0, start=True, stop=True)
            nc.tensor.matmul(c1[:], ones_col[:], state1, start=True, stop=True)
            return c0, c1

        def extract_out(c0, c1, row_sb, row_out, copy_eng):
            if copy_eng == "act":
                nc.scalar.copy(row_sb[0:1, 0:G0W], c0[:])
                nc.scalar.copy(row_sb[0:1, G0W:W], c1[:])
            else:  # tail: run the two copies on parallel engines
                nc.vector.tensor_copy(row_sb[0:1, 0:G0W], c0[:])
                nc.scalar.copy(row_sb[0:1, G0W:W], c1[:])
            nc.sync.dma_start(row_out[:], row_sb[:])

        # ---- the scan ---------------------------------------------------
        # state_0 = E at each chain's local step 0, read in place
        st0 = None   # group tiles; step 1 reads E directly
        st1 = None
        mm1 = prev_mm1 = None
        for kk in range(1, L):
            pp0 = ps.tile([C, G0W], F32, tag="pp0", bufs=2, name=f"pp0_{kk}")
            pp1 = ps.tile([C, G1W], F32, tag="pp1", bufs=2, name=f"pp1_{kk}")
            if kk == 1:
                # chain 0 inits exactly from E_0; chains j>0 warm-start
                # from ones (only the direction must converge)
                nc.tensor.matmul(pp0[:, 0:BC], wf[:], E[:, 0:BC],
                                 start=True, stop=True)
                nc.tensor.matmul(pp0[:, BC:G0W], wf[:],
                                 ones_mv[:, 0:G0W - BC],
                                 start=True, stop=True)
                nc.tensor.matmul(pp1[:], wf[:], ones_mv[:],
                                 start=True, stop=True)
            else:
                nc.tensor.matmul(pp0[:], wf[:], st0, start=True, stop=True)
                prev_mm1 = mm1
                mm1 = nc.tensor.matmul(pp1[:], wf[:], st1,
                                       start=True, stop=True)

            a0 = wk.tile([C, G0W], BF16, tag="a0", bufs=3, name=f"a0_{kk}")
            a1 = wk.tile([C, G1W], BF16, tag="a1", bufs=3, name=f"a1_{kk}")
            o0, n0 = e_rng(kk, 0)
            o1, n1 = e_rng(kk, 1)
            mi0 = nc.vector.tensor_tensor(a0[:], pp0[:], E[:, o0:o0 + n0],
                                          op=OP.mult)
            nc.vector.tensor_tensor(a1[:], pp1[:], E[:, o1:o1 + n1],
                                    op=OP.mult)
            st0, st1 = a0[:], a1[:]

            if kk == O:
                cso_ps = extract_mm(st0, st1)
            if kk == 20:
                # the PSUM->SBUF copies run here, when ACT is done with exp
                extract_out(cso_ps[0], cso_ps[1], cso_sb, cso_out, "act")

            # stream exp 2+ rounds ahead of consumption; split the early
            # chunks at the group boundary so group 0 unblocks sooner
            if 1 <= kk <= 3:
                exp_chunk(kk + 1, split=G0W)
            elif kk == 4:
                exp_chunk(5, split=G0W)
                exp_chunk(6, split=W)
            elif kk == 5:
                exp_chunk(7)
            elif kk == 6:
                exp_chunk(8)
            elif kk == 10:
                # chunk 0's remainder (block DE): first re-read at round DE
                nc.scalar.activation(E[:, BC:W + BC], raw[:, BC:W + BC],
                                     AF.Exp, bias=bias[:])
                exp_chunk(11)
            elif kk in (8, 12, 14, 16):
                exp_chunk({8: 9, 12: 10, 14: 12, 16: 13}[kk])
            if kk == 14:
                gold_trans()
            # gold matmuls ride the idle PE slots once hemit has landed
            if kk >= 10:
                gold_mm(7, prev_mm1)

        gold_mm(NGC if _EN_GOLD else 0)   # any leftovers
        gold_finish(mi0)                  # ready before the final states
        cf = extract_mm(st0, st1)
        extract_out(cf[0], cf[1], csf_sb, csf_out, "split")

    nc.compile()
    return nc


# stored column -> (batch row, time) maps, shared by et and hemit prep
_COL_B = np.empty(NCOL, dtype=np.int64)
_COL_T = np.empty(NCOL, dtype=np.int64)
for _k in _ORDER:
    if _k <= O:
        _sl = slice(OFF[_k], OFF[_k] + BC)
        _COL_B[_sl] = np.arange(BC)
        _COL_T[_sl] = _k
    else:
        _sl = slice(OFF[_k], OFF[_k] + W)
        _COL_B[_sl] = np.tile(np.arange(BC), K)
        _COL_T[_sl] = np.repeat(np.arange(K) * DE + _k, BC)


def _prep_inputs(emissions, tags, mask, transitions):
    em = np.asarray(emissions, dtype=np.float32)
    tg = np.asarray(tags).astype(np.int64)
    mk = np.asarray(mask).astype(np.float32)
    tr = np.ascontiguousarray(np.asarray(transitions, dtype=np.float32))

    afwd = np.exp(tr.astype(np.float64)).astype(ml_dtypes.bfloat16)
    ident = np.eye(C, dtype=ml_dtypes.bfloat16)

    in_maps = []
    for core in range(NCORES):
        b0 = core * BC
        ec = em[b0:b0 + BC]                        # [BC,S,C]
        ett = ec.transpose(2, 1, 0)                # [C,S,BC]
        et = np.ascontiguousarray(
            ett[:, _COL_T, _COL_B]).astype(ml_dtypes.float8_e4m3fn)

        tgc = tg[b0:b0 + BC]
        mkc = mk[b0:b0 + BC]
        hemit = np.zeros((C, NCOL), dtype=ml_dtypes.float8_e4m3fn)
        hemit[tgc[_COL_B, _COL_T], np.arange(NCOL)] = \
            mkc[_COL_B, _COL_T].astype(ml_dtypes.float8_e4m3fn)

        cnt = np.zeros((C, C), dtype=np.float64)
        np.add.at(cnt, (tgc[:, :-1].ravel(), tgc[:, 1:].ravel()),
                  mkc[:, 1:].ravel().astype(np.float64))
        cnt = cnt.astype(np.float32)

        in_maps.append({
            "et": et, "hemit": hemit, "afwd": afwd,
            "cnt": cnt, "tsb": tr, "ident": ident,
        })
    return in_maps


def kernel(emissions, tags, mask, transitions, _trace=False):
    global _NC_CACHE
    if _NC_CACHE is None:
        _NC_CACHE = _build_nc()
    nc = _NC_CACHE

    in_maps = _prep_inputs(emissions, tags, mask, transitions)
    res = run_bass_kernel_spmd(
        nc, in_maps, core_ids=list(range(NCORES)), trace=_trace,
    )
    partition = np.float64(0.0)
    gold = np.float64(0.0)
    for r in res.results:
        n = np.asarray(r["csf"], dtype=np.float64).reshape(K, BC)
        g = np.asarray(r["cso"], dtype=np.float64).reshape(K, BC)
        logZ = np.log(n[K - 1]) + MU * S
        logZ += (np.log(n[:K - 1]) - np.log(g[1:])).sum(axis=0)
        partition += logZ.sum()
        gold += np.asarray(r["gold"], dtype=np.float64).sum()
    out = np.float32(partition - gold)
    if _trace:
        return out, res
    return out


# revision 59
# speedup vs baseline: 1.0034x; 1.0034x over previous
"""CRF negative-log-likelihood kernel for Trainium2 (8 NeuronCores).

Math: reference computes  partition - gold  where
  partition = sum_b logsumexp_c(alpha[511])  via the forward algorithm
  gold      = sum emissions[b,s,tags] * m + sum T[tags[s],tags[s+1]] * m[:,1:]

Device strategy (data-parallel over batch, 32 rows per core):
  * Linear domain: alpha_t = E_t o (A^T alpha_{t-1}) with A = exp(T),
    E_t = exp(e_t - MU).  One [128,128]x[128,W] matmul (PE) plus one
    elementwise multiply (DVE) per step.
  * K=23 overlapping forward chains cut the serial depth from 511 steps
    to L-1=27.  Chain j starts at t = j*DELTA from the raw emission
    vector E_{j*DELTA} and runs L=28 steps; its first O=5 steps are
    warm-up inside chain j-1's range.  Products of >=5 random positive
    matrices are numerically rank-1 (Perron-Frobenius contraction), so
    the chains glue exactly through two column-sum scalars per junction:
      logZ_b = log n[K-1] + sum_j (log n[j-1] - log gamma[j]) + MU*S
    where gamma[j] = colsum of chain j's state after its warm-up step O
    and n[j] = colsum at its final step (both measure t = j*DELTA + O
    resp. j*DELTA + L-1; the grid aligns junctions exactly).  Host takes
    the logs in f64.  Validated: junction error ~1e-16, total loss
    rel err ~3e-5 (bf16/fp8 rounding dominated).
  * No renormalisation: the exp bias -MU keeps per-step growth ~1, and
    a 28-step chain drifts far less than the f32/bf16 exponent range.
  * The scan runs as G=2 independent chain-groups (12+11 chains wide)
    round-robined so the DVE (the bottleneck engine: 125ns PSUM-access
    init + 1.04ns/col) stays saturated while semaphore round-trips hide.
  * Emissions arrive as fp8-e4m3 (halves DMA; validated noise ~1e-4) in
    a step-major block layout so DMA+exp stream strictly ahead of
    consumption and every scan-step read is one contiguous slice.
  * Gold emit: sum(raw o onehot(tags)) via PE: 128 PSUM-accumulated
    fp8 matmuls H_c^T R_c (diag trick), injected into scan-idle PE
    slots; diag extracted with an identity multiply + free-axis reduce.
  * Gold trans: host-built pair-count matrix CNT (index-only prep),
    mul+reduce against T on Pool/DVE.
  * Startup/tail: activation-table load and PE p-state ramp pre-warmed
    under the DMA shadow; weights on the parallel Pool DMA queue; gold
    reduction finishes on Pool so its DMA overlaps the csf output path.
Outputs per core: two colsum rows + gold column; host sums in float64.
"""

import sys

for _p in ("/opt/trn_rl_repo",):
    if _p not in sys.path:
        sys.path.insert(0, _p)

import os as _os
import numpy as np
import ml_dtypes
from contextlib import ExitStack

from concourse import bass, tile, mybir, bacc
from concourse.bass_utils import run_bass_kernel_spmd

NCORES = 8
B, S, C = 256, 512, 128
BC = B // NCORES          # batch rows per core
K = 23                    # chains
O = 5                     # warm-up steps per chain
DE = 22                   # chain start stride (DELTA)
L = DE + O + 1            # steps per chain (incl. init step 0)
MU = 5.85                 # exp prescale; host adds MU*S back per batch row
W = K * BC                # 736: full state width
G0W = 12 * BC             # group 0: chains 0..11  (384 cols)
G1W = 11 * BC             # group 1: chains 12..22 (352 cols)
NCOL = S * BC             # 16384 stored emission columns per core
assert K * DE == S - 1 - O and (K - 1) * DE + L - 1 == S - 1

# stored block order = consumption order: small block BLK(k) (chain 0's
# 32-col tile for t=k) directly before big block BLK(k+DE) for k=0..O,
# then big BLK(O+1..DE-1).  BLK(k>O) holds slot j = chain j's tile for
# t = j*DE + k (K*32 cols).  Small-before-big makes every warm-up read
# [chain0 | chains 1..11] one contiguous 384-col slice.
_ORDER = []
for _k in range(O + 1):
    _ORDER += [_k, DE + _k]
_ORDER += list(range(O + 1, DE))
OFF = {}
_pos = 0
for _k in _ORDER:
    OFF[_k] = _pos
    _pos += W if _k > O else BC
assert _pos == NCOL

# exp chunks: (offset, size) pairs in stored order.  Chunk c<=5 feeds
# scan step c (and steps 22..27); bulk chunk 6+i (2 blocks) feeds steps
# 6+2i and 7+2i.
CHUNKS = []
for _i in range(O + 1):                       # 6 chunks of 768
    CHUNKS.append((_i * (W + BC), W + BC))
_base = (O + 1) * (W + BC)
for _i in range(8):                           # 8 chunks of 2*736
    CHUNKS.append((_base + _i * 2 * W, 2 * W))
assert CHUNKS[-1][0] + CHUNKS[-1][1] == NCOL

# DMA batches (HWDGE costs 625ns per dma_start; batch the tail, but keep
# the early chunks separate so each lands — and unblocks its exp — ASAP).
# Chains j>0 warm-start from ones, so round 0 needs only the 32-col s0
# slice; block DE+0 (chunk 0's remainder) isn't read again until round
# DE, letting its DMA+exp drop out of the startup chain entirely.
_D1 = 3 * (W + BC)
DMA_BATCHES = [CHUNKS[1], (0, BC), CHUNKS[2],
               (_D1, _base - _D1), (_base, 2 * W),
               (_base + 2 * W, NCOL - _base - 2 * W), (BC, W)]

F32 = mybir.dt.float32
BF16 = mybir.dt.bfloat16
FP8 = mybir.dt.float8e4
AF = mybir.ActivationFunctionType
OP = mybir.AluOpType

_EN_GOLD = _os.environ.get("CRF_GOLD", "1") == "1"
_EN_SCAN = _os.environ.get("CRF_SCAN", "1") == "1"

_NC_CACHE = None


def _build_nc():
    nc = bacc.Bacc("TRN2", target_bir_lowering=False, debug=False)

    et_in = nc.dram_tensor("et", [C, NCOL], FP8, kind="ExternalInput").ap()
    hemit_in = nc.dram_tensor("hemit", [C, NCOL], FP8,
                              kind="ExternalInput").ap()
    afwd = nc.dram_tensor("afwd", [C, C], BF16, kind="ExternalInput").ap()
    cnt_in = nc.dram_tensor("cnt", [C, C], F32, kind="ExternalInput").ap()
    tsb_in = nc.dram_tensor("tsb", [C, C], F32, kind="ExternalInput").ap()
    id_in = nc.dram_tensor("ident", [C, C], BF16, kind="ExternalInput").ap()
    cso_out = nc.dram_tensor("cso", [1, W], F32, kind="ExternalOutput").ap()
    csf_out = nc.dram_tensor("csf", [1, W], F32, kind="ExternalOutput").ap()
    gold = nc.dram_tensor("gold", [C, 1], F32, kind="ExternalOutput").ap()

    with tile.TileContext(nc) as tc, ExitStack() as ctx:
        sb = ctx.enter_context(tc.tile_pool(name="sb", bufs=1))
        wk = ctx.enter_context(tc.tile_pool(name="wk", bufs=4))
        ps = ctx.enter_context(tc.tile_pool(name="ps", bufs=2, space="PSUM"))

        # ---- persistent tiles -------------------------------------------
        bias = sb.tile([C, 1], F32, name="bias")
        nc.vector.memset(bias[:], -MU)
        ones_col = sb.tile([C, 1], BF16, name="ones_col")
        nc.vector.memset(ones_col[:], 1.0)
        # dummy exp: pulls the 1283ns activation-table load into the DMA
        # shadow at t=0
        warm = sb.tile([C, 1], BF16, name="warm")
        nc.scalar.activation(warm[:], bias[:], AF.Exp, bias=bias[:])
        # dummy matmul: starts the PE p-state ramp clock (3us to full
        # speed) during the DMA shadow so round 0 runs at full clock
        pwarm = ps.tile([1, 1], F32, tag="pw", bufs=1, name="pwarm")
        nc.tensor.matmul(pwarm[:], ones_col[:], ones_col[:],
                         start=True, stop=True)

        raw = sb.tile([C, NCOL], FP8, name="raw")
        E = sb.tile([C, NCOL], BF16, name="E")
        hem = sb.tile([C, NCOL], FP8, name="hem")
        wf = sb.tile([C, C], BF16, name="wf")
        cso_sb = sb.tile([1, W], F32, name="cso_sb")
        csf_sb = sb.tile([1, W], F32, name="csf_sb")

        # ---- input DMA: et batches in consumption order (weights after
        # the early batches: not needed until the first matmul), then
        # hemit (needed mid-scan for gold), then the small gold inputs ----
        # weights ride the otherwise-idle Pool SWDGE queue, in parallel
        # with the emission stream on the SP queue
        nc.gpsimd.dma_start(wf[:], afwd[:])
        for o, n in DMA_BATCHES:
            nc.sync.dma_start(raw[:, o:o + n], et_in[:, o:o + n])
        hq = NCOL // 2
        for i in range(2):
            nc.sync.dma_start(hem[:, i * hq:(i + 1) * hq],
                              hemit_in[:, i * hq:(i + 1) * hq])
        cnt_sb = sb.tile([C, C], F32, name="cnt_sb")
        tsb = sb.tile([C, C], F32, name="tsb_t")
        ident = sb.tile([C, C], BF16, name="ident")
        nc.sync.dma_start(cnt_sb[:], cnt_in[:])
        nc.sync.dma_start(tsb[:], tsb_in[:])
        nc.sync.dma_start(ident[:], id_in[:])

        def exp_chunk(c, split=0):
            o, n = CHUNKS[c]
            if split:
                nc.scalar.activation(E[:, o:o + split], raw[:, o:o + split],
                                     AF.Exp, bias=bias[:])
                nc.scalar.activation(E[:, o + split:o + n],
                                     raw[:, o + split:o + n],
                                     AF.Exp, bias=bias[:])
            else:
                nc.scalar.activation(E[:, o:o + n], raw[:, o:o + n], AF.Exp,
                                     bias=bias[:])

        # E source slice for (step, group).  Small-before-big block order
        # makes group 0's warm-up read [chain0 | big-block slots 0..10]
        # contiguous, so both groups always read one slice starting at
        # OFF[kk] (the small block for kk<=O, the big block otherwise).
        def e_rng(kk, grp):
            if grp == 0:
                return OFF[kk], G0W
            return OFF[kk] + G0W, G1W

        # gold state
        if _EN_GOLD:
            gold_ps = ps.tile([C, C], F32, tag="gps", bufs=1, name="gold_ps")
            NGC = NCOL // C                            # 128 matmul chunks
            gpos = [0]
            ttr = sb.tile([C, C], F32, name="ttr")

            def gold_trans():
                # cnt o T multiply on Pool (idle during the scan)
                nc.gpsimd.tensor_tensor(ttr[:], cnt_sb[:], tsb[:], op=OP.mult)

            def gold_mm(nmm, anchor=None):
                from concourse.tile_rust import add_dep_helper
                for i in range(nmm):
                    m = gpos[0]
                    if m >= NGC:
                        return
                    gpos[0] += 1
                    gi = nc.tensor.matmul(
                        gold_ps[:], hem[:, m * C:(m + 1) * C],
                        raw[:, m * C:(m + 1) * C],
                        start=(m == 0), stop=(m == NGC - 1))
                    if i == 0 and anchor is not None:
                        # pin the batch into this round's PE idle window --
                        # Tile otherwise hoists it between the scan matmuls
                        add_dep_helper(gi.ins, anchor.ins,
                                       reason="gold batch after scan matmul")

            def gold_finish(anchor=None):
                from concourse.tile_rust import add_dep_helper
                gacc = sb.tile([C, 1], F32, name="gacc")
                tp = sb.tile([C, 1], F32, name="tp")
                trash = sb.tile([C, C], BF16, name="trash")
                nc.vector.tensor_tensor(trash[:], gold_ps[:], ident[:],
                                        op=OP.mult)
                nc.vector.reduce_sum(gacc[:], trash[:],
                                     axis=mybir.AxisListType.X)
                rtp = nc.vector.reduce_sum(tp[:], ttr[:],
                                           axis=mybir.AxisListType.X)
                if anchor is not None:
                    # keep this off the saturated DVE until the scan is done
                    # (its inputs are ready early and Tile hoists it)
                    add_dep_helper(rtp.ins, anchor.ins,
                                   reason="gold reduce after last scan mult")
                # final add + DMA on the idle Pool engine/queue so the gold
                # path never queues behind the csf copies on DVE
                nc.gpsimd.tensor_add(gacc[:], gacc[:], tp[:])
                nc.gpsimd.dma_start(gold[:], gacc[:])
        else:
            def gold_trans():
                pass

            def gold_mm(nmm, anchor=None):
                pass

            def gold_finish(anchor=None):
                zg = sb.tile([C, 1], F32, name="zg")
                nc.vector.memset(zg[:], 0.0)
                nc.sync.dma_start(gold[:], zg[:])

        if not _EN_SCAN:
            zr = sb.tile([1, W], F32, name="zr")
            nc.vector.memset(zr[:], 1.0)
            nc.sync.dma_start(cso_out[:], zr[:])
            nc.sync.dma_start(csf_out[:], zr[:])
            for c in range(len(CHUNKS)):
                exp_chunk(c)
            gold_trans()
            gold_mm(NCOL // C if _EN_GOLD else 0)
            gold_finish()
            nc.compile()
            return nc

        # exp the slices the first scan step needs, smallest-first so each
        # consumer (init matmuls, then the two step-1 mults) starts ASAP
        o1, n1 = CHUNKS[1]
        nc.scalar.activation(E[:, 0:BC], raw[:, 0:BC], AF.Exp, bias=bias[:])
        nc.scalar.activation(E[:, o1:o1 + G0W], raw[:, o1:o1 + G0W],
                             AF.Exp, bias=bias[:])
        nc.scalar.activation(E[:, o1 + G0W:o1 + n1], raw[:, o1 + G0W:o1 + n1],
                             AF.Exp, bias=bias[:])
        ones_mv = sb.tile([C, G1W], BF16, name="ones_mv")
        nc.vector.memset(ones_mv[:], 1.0)

        def extract_mm(state0, state1):
            # colsums via ones-matmul into PSUM (held there until copied)
            c0 = ps.tile([1, G0W], F32, tag="cs0", bufs=1, name="c0")
            c1 = ps.tile([1, G1W], F32, tag="cs1", bufs=1, name="c1")
            nc.tensor.matmul(c0[:], ones_col[:], state0, start=True, stop=True)
            nc.tensor.matmul(c1[:], ones_col[:], state1, start=True, stop=True)
            return c0, c1

        def extract_out(c0, c1, row_sb, row_out, copy_eng):
            if copy_eng == "act":
                nc.scalar.copy(row_sb[0:1, 0:G0W], c0[:])
                nc.scalar.copy(row_sb[0:1, G0W:W], c1[:])
            else:  # tail: run the two copies on parallel engines
                nc.vector.tensor_copy(row_sb[0:1, 0:G0W], c0[:])
                nc.scalar.copy(row_sb[0:1, G0W:W], c1[:])
            nc.sync.dma_start(row_out[:], row_sb[:])

        # ---- the scan ---------------------------------------------------
        # state_0 = E at each chain's local step 0, read in place
        st0 = None   # group tiles; step 1 reads E directly
        st1 = None
        mm1 = prev_mm1 = None
        for kk in range(1, L):
            pp0 = ps.tile([C, G0W], F32, tag="pp0", bufs=2, name=f"pp0_{kk}")
            pp1 = ps.tile([C, G1W], F32, tag="pp1", bufs=2, name=f"pp1_{kk}")
            if kk == 1:
                # chain 0 inits exactly from E_0; chains j>0 warm-start
                # from ones (only the direction must converge)
                nc.tensor.matmul(pp0[:, 0:BC], wf[:], E[:, 0:BC],
                                 start=True, stop=True)
                nc.tensor.matmul(pp0[:, BC:G0W], wf[:],
                                 ones_mv[:, 0:G0W - BC],
                                 start=True, stop=True)
                nc.tensor.matmul(pp1[:], wf[:], ones_mv[:],
                                 start=True, stop=True)
            else:
                nc.tensor.matmul(pp0[:], wf[:], st0, start=True, stop=True)
                prev_mm1 = mm1
                mm1 = nc.tensor.matmul(pp1[:], wf[:], st1,
                                       start=True, stop=True)

            a0 = wk.tile([C, G0W], BF16, tag="a0", bufs=3, name=f"a0_{kk}")
            a1 = wk.tile([C, G1W], BF16, tag="a1", bufs=3, name=f"a1_{kk}")
            o0, n0 = e_rng(kk, 0)
            o1, n1 = e_rng(kk, 1)
            mi0 = nc.vector.tensor_tensor(a0[:], pp0[:], E[:, o0:o0 + n0],
                                          op=OP.mult)
            nc.vector.tensor_tensor(a1[:], pp1[:], E[:, o1:o1 + n1],
                                    op=OP.mult)
            st0, st1 = a0[:], a1[:]

            if kk == O:
                cso_ps = extract_mm(st0, st1)
            if kk == 20:
                # the PSUM->SBUF copies run here, when ACT is done with exp
                extract_out(cso_ps[0], cso_ps[1], cso_sb, cso_out, "act")

            # stream exp 2+ rounds ahead of consumption
            if 1 <= kk <= 3:
                exp_chunk(kk + 1)
            elif kk == 4:
                exp_chunk(5)
                exp_chunk(6)
            elif kk == 5:
                exp_chunk(7)
            elif kk == 6:
                exp_chunk(8)
            elif kk == 10:
                # chunk 0's remainder (block DE): first re-read at round DE
                nc.scalar.activation(E[:, BC:W + BC], raw[:, BC:W + BC],
                                     AF.Exp, bias=bias[:])
                exp_chunk(11)
            elif kk in (8, 12, 14, 16):
                exp_chunk({8: 9, 12: 10, 14: 12, 16: 13}[kk])
            if kk == 14:
                gold_trans()
            # gold matmuls ride the idle PE slots once hemit has landed
            if kk >= 10:
                gold_mm(7, prev_mm1)

        gold_mm(NGC if _EN_GOLD else 0)   # any leftovers
        gold_finish(mi0)                  # ready before the final states
        cf = extract_mm(st0, st1)
        extract_out(cf[0], cf[1], csf_sb, csf_out, "split")

    nc.compile()
    return nc


# stored column -> (batch row, time) maps, shared by et and hemit prep
_COL_B = np.empty(NCOL, dtype=np.int64)
_COL_T = np.empty(NCOL, dtype=np.int64)
for _k in _ORDER:
    if _k <= O:
        _sl = slice(OFF[_k], OFF[_k] + BC)
        _COL_B[_sl] = np.arange(BC)
        _COL_T[_sl] = _k
    else:
        _sl = slice(OFF[_k], OFF[_k] + W)
        _COL_B[_sl] = np.tile(np.arange(BC), K)
        _COL_T[_sl] = np.repeat(np.arange(K) * DE + _k, BC)


def _prep_inputs(emissions, tags, mask, transitions):
    em = np.asarray(emissions, dtype=np.float32)
    tg = np.asarray(tags).astype(np.int64)
    mk = np.asarray(mask).astype(np.float32)
    tr = np.ascontiguousarray(np.asarray(transitions, dtype=np.float32))

    afwd = np.exp(tr.astype(np.float64)).astype(ml_dtypes.bfloat16)
    ident = np.eye(C, dtype=ml_dtypes.bfloat16)

    in_maps = []
    for core in range(NCORES):
        b0 = core * BC
        ec = em[b0:b0 + BC]                        # [BC,S,C]
        ett = ec.transpose(2, 1, 0)                # [C,S,BC]
        et = np.ascontiguousarray(
            ett[:, _COL_T, _COL_B]).astype(ml_dtypes.float8_e4m3fn)

        tgc = tg[b0:b0 + BC]
        mkc = mk[b0:b0 + BC]
        hemit = np.zeros((C, NCOL), dtype=ml_dtypes.float8_e4m3fn)
        hemit[tgc[_COL_B, _COL_T], np.arange(NCOL)] = \
            mkc[_COL_B, _COL_T].astype(ml_dtypes.float8_e4m3fn)

        cnt = np.zeros((C, C), dtype=np.float64)
        np.add.at(cnt, (tgc[:, :-1].ravel(), tgc[:, 1:].ravel()),
                  mkc[:, 1:].ravel().astype(np.float64))
        cnt = cnt.astype(np.float32)

        in_maps.append({
            "et": et, "hemit": hemit, "afwd": afwd,
            "cnt": cnt, "tsb": tr, "ident": ident,
        })
    return in_maps


def kernel(emissions, tags, mask, transitions, _trace=False):
    global _NC_CACHE
    if _NC_CACHE is None:
        _NC_CACHE = _build_nc()
    nc = _NC_CACHE

    in_maps = _prep_inputs(emissions, tags, mask, transitions)
    res = run_bass_kernel_spmd(
        nc, in_maps, core_ids=list(range(NCORES)), trace=_trace,
    )
    partition = np.float64(0.0)
    gold = np.float64(0.0)
    for r in res.results:
        n = np.asarray(r["csf"], dtype=np.float64).reshape(K, BC)
        g = np.asarray(r["cso"], dtype=np.float64).reshape(K, BC)
        logZ = np.log(n[K - 1]) + MU * S
        logZ += (np.log(n[:K - 1]) - np.log(g[1:])).sum(axis=0)
        partition += logZ.sum()
        gold += np.asarray(r["gold"], dtype=np.float64).sum()
    out = np.float32(partition - gold)
    if _trace:
        return out, res
    return out


# revision 60
# speedup vs baseline: 1.0143x; 1.0108x over previous
"""CRF negative-log-likelihood kernel for Trainium2 (8 NeuronCores).

Math: reference computes  partition - gold  where
  partition = sum_b logsumexp_c(alpha[511])  via the forward algorithm
  gold      = sum emissions[b,s,tags] * m + sum T[tags[s],tags[s+1]] * m[:,1:]

Device strategy (data-parallel over batch, 32 rows per core):
  * Linear domain: alpha_t = E_t o (A^T alpha_{t-1}) with A = exp(T),
    E_t = exp(e_t - MU).  One [128,128]x[128,W] matmul (PE) plus one
    elementwise multiply (DVE) per step.
  * K=23 overlapping forward chains cut the serial depth from 511 steps
    to L-1=27.  Chain j starts at t = j*DELTA from the raw emission
    vector E_{j*DELTA} and runs L=28 steps; its first O=5 steps are
    warm-up inside chain j-1's range.  Products of >=5 random positive
    matrices are numerically rank-1 (Perron-Frobenius contraction), so
    the chains glue exactly through two column-sum scalars per junction:
      logZ_b = log n[K-1] + sum_j (log n[j-1] - log gamma[j]) + MU*S
    where gamma[j] = colsum of chain j's state after its warm-up step O
    and n[j] = colsum at its final step (both measure t = j*DELTA + O
    resp. j*DELTA + L-1; the grid aligns junctions exactly).  Host takes
    the logs in f64.  Validated: junction error ~1e-16, total loss
    rel err ~3e-5 (bf16/fp8 rounding dominated).
  * No renormalisation: the exp bias -MU keeps per-step growth ~1, and
    a 28-step chain drifts far less than the f32/bf16 exponent range.
  * The scan runs as G=2 independent chain-groups (12+11 chains wide)
    round-robined so the DVE (the bottleneck engine: 125ns PSUM-access
    init + 1.04ns/col) stays saturated while semaphore round-trips hide.
  * Emissions arrive as fp8-e4m3 (halves DMA; validated noise ~1e-4) in
    a step-major block layout so DMA+exp stream strictly ahead of
    consumption and every scan-step read is one contiguous slice.
  * Gold emit: sum(raw o onehot(tags)) via PE: 128 PSUM-accumulated
    fp8 matmuls H_c^T R_c (diag trick), injected into scan-idle PE
    slots; diag extracted with an identity multiply + free-axis reduce.
  * Gold trans: host-built pair-count matrix CNT (index-only prep),
    mul+reduce against T on Pool/DVE.
  * Startup/tail: activation-table load and PE p-state ramp pre-warmed
    under the DMA shadow; weights on the parallel Pool DMA queue; gold
    reduction finishes on Pool so its DMA overlaps the csf output path.
Outputs per core: two colsum rows + gold column; host sums in float64.
"""

import sys

for _p in ("/opt/trn_rl_repo",):
    if _p not in sys.path:
        sys.path.insert(0, _p)

import os as _os
import numpy as np
import ml_dtypes
from contextlib import ExitStack

from concourse import bass, tile, mybir, bacc
from concourse.bass_utils import run_bass_kernel_spmd

NCORES = 8
B, S, C = 256, 512, 128
BC = B // NCORES          # batch rows per core
K = 23                    # chains
O = 5                     # warm-up steps per chain
DE = 22                   # chain start stride (DELTA)
L = DE + O + 1            # steps per chain (incl. init step 0)
MU = 5.85                 # exp prescale; host adds MU*S back per batch row
W = K * BC                # 736: full state width
G0W = 12 * BC             # group 0: chains 0..11  (384 cols)
G1W = 11 * BC             # group 1: chains 12..22 (352 cols)
NCOL = S * BC             # 16384 stored emission columns per core
assert K * DE == S - 1 - O and (K - 1) * DE + L - 1 == S - 1

# stored block order = consumption order: small block BLK(k) (chain 0's
# 32-col tile for t=k) directly before big block BLK(k+DE) for k=0..O,
# then big BLK(O+1..DE-1).  BLK(k>O) holds slot j = chain j's tile for
# t = j*DE + k (K*32 cols).  Small-before-big makes every warm-up read
# [chain0 | chains 1..11] one contiguous 384-col slice.
_ORDER = []
for _k in range(O + 1):
    _ORDER += [_k, DE + _k]
_ORDER += list(range(O + 1, DE))
OFF = {}
_pos = 0
for _k in _ORDER:
    OFF[_k] = _pos
    _pos += W if _k > O else BC
assert _pos == NCOL

# exp chunks: (offset, size) pairs in stored order.  Chunk c<=5 feeds
# scan step c (and steps 22..27); bulk chunk 6+i (2 blocks) feeds steps
# 6+2i and 7+2i.
CHUNKS = []
for _i in range(O + 1):                       # 6 chunks of 768
    CHUNKS.append((_i * (W + BC), W + BC))
_base = (O + 1) * (W + BC)
for _i in range(8):                           # 8 chunks of 2*736
    CHUNKS.append((_base + _i * 2 * W, 2 * W))
assert CHUNKS[-1][0] + CHUNKS[-1][1] == NCOL

# DMA batches (HWDGE costs 625ns per dma_start; batch the tail, but keep
# the early chunks separate so each lands — and unblocks its exp — ASAP).
# Chains j>0 warm-start from ones, so round 0 needs only the 32-col s0
# slice; block DE+0 (chunk 0's remainder) isn't read again until round
# DE, letting its DMA+exp drop out of the startup chain entirely.
_D1 = 3 * (W + BC)
DMA_BATCHES = [CHUNKS[1], (0, BC), CHUNKS[2],
               (_D1, _base - _D1), (_base, 2 * W),
               (_base + 2 * W, NCOL - _base - 2 * W), (BC, W)]

F32 = mybir.dt.float32
BF16 = mybir.dt.bfloat16
FP8 = mybir.dt.float8e4
AF = mybir.ActivationFunctionType
OP = mybir.AluOpType

_EN_GOLD = _os.environ.get("CRF_GOLD", "1") == "1"
_EN_SCAN = _os.environ.get("CRF_SCAN", "1") == "1"

_NC_CACHE = None


def _build_nc():
    nc = bacc.Bacc("TRN2", target_bir_lowering=False, debug=False)

    et_in = nc.dram_tensor("et", [C, NCOL], FP8, kind="ExternalInput").ap()
    hemit_in = nc.dram_tensor("hemit", [C, NCOL], FP8,
                              kind="ExternalInput").ap()
    afwd = nc.dram_tensor("afwd", [C, C], BF16, kind="ExternalInput").ap()
    cnt_in = nc.dram_tensor("cnt", [C, C], F32, kind="ExternalInput").ap()
    tsb_in = nc.dram_tensor("tsb", [C, C], F32, kind="ExternalInput").ap()
    id_in = nc.dram_tensor("ident", [C, C], BF16, kind="ExternalInput").ap()
    cso_out = nc.dram_tensor("cso", [1, W], F32, kind="ExternalOutput").ap()
    csf_out = nc.dram_tensor("csf", [1, W], F32, kind="ExternalOutput").ap()
    gold = nc.dram_tensor("gold", [C, 2 * C], BF16,
                          kind="ExternalOutput").ap()

    with tile.TileContext(nc) as tc, ExitStack() as ctx:
        sb = ctx.enter_context(tc.tile_pool(name="sb", bufs=1))
        wk = ctx.enter_context(tc.tile_pool(name="wk", bufs=4))
        ps = ctx.enter_context(tc.tile_pool(name="ps", bufs=2, space="PSUM"))

        # ---- persistent tiles -------------------------------------------
        bias = sb.tile([C, 1], F32, name="bias")
        nc.vector.memset(bias[:], -MU)
        ones_col = sb.tile([C, 1], BF16, name="ones_col")
        nc.vector.memset(ones_col[:], 1.0)
        # dummy exp: pulls the 1283ns activation-table load into the DMA
        # shadow at t=0
        warm = sb.tile([C, 1], BF16, name="warm")
        nc.scalar.activation(warm[:], bias[:], AF.Exp, bias=bias[:])
        # dummy matmul: starts the PE p-state ramp clock (3us to full
        # speed) during the DMA shadow so round 0 runs at full clock
        pwarm = ps.tile([1, 1], F32, tag="pw", bufs=1, name="pwarm")
        nc.tensor.matmul(pwarm[:], ones_col[:], ones_col[:],
                         start=True, stop=True)

        raw = sb.tile([C, NCOL], FP8, name="raw")
        E = sb.tile([C, NCOL], BF16, name="E")
        hem = sb.tile([C, NCOL], FP8, name="hem")
        wf = sb.tile([C, C], BF16, name="wf")
        cso_sb = sb.tile([1, W], F32, name="cso_sb")
        csf_sb = sb.tile([1, W], F32, name="csf_sb")

        # ---- input DMA: et batches in consumption order (weights after
        # the early batches: not needed until the first matmul), then
        # hemit (needed mid-scan for gold), then the small gold inputs ----
        # weights ride the otherwise-idle Pool SWDGE queue, in parallel
        # with the emission stream on the SP queue
        nc.gpsimd.dma_start(wf[:], afwd[:])
        for o, n in DMA_BATCHES:
            nc.sync.dma_start(raw[:, o:o + n], et_in[:, o:o + n])
        hq = NCOL // 2
        for i in range(2):
            nc.sync.dma_start(hem[:, i * hq:(i + 1) * hq],
                              hemit_in[:, i * hq:(i + 1) * hq])
        cnt_sb = sb.tile([C, C], F32, name="cnt_sb")
        tsb = sb.tile([C, C], F32, name="tsb_t")
        ident = sb.tile([C, C], BF16, name="ident")
        nc.sync.dma_start(cnt_sb[:], cnt_in[:])
        nc.sync.dma_start(tsb[:], tsb_in[:])
        nc.sync.dma_start(ident[:], id_in[:])

        def exp_chunk(c, split=0):
            o, n = CHUNKS[c]
            if split:
                nc.scalar.activation(E[:, o:o + split], raw[:, o:o + split],
                                     AF.Exp, bias=bias[:])
                nc.scalar.activation(E[:, o + split:o + n],
                                     raw[:, o + split:o + n],
                                     AF.Exp, bias=bias[:])
            else:
                nc.scalar.activation(E[:, o:o + n], raw[:, o:o + n], AF.Exp,
                                     bias=bias[:])

        # E source slice for (step, group).  Small-before-big block order
        # makes group 0's warm-up read [chain0 | big-block slots 0..10]
        # contiguous, so both groups always read one slice starting at
        # OFF[kk] (the small block for kk<=O, the big block otherwise).
        def e_rng(kk, grp):
            if grp == 0:
                return OFF[kk], G0W
            return OFF[kk] + G0W, G1W

        # gold state
        if _EN_GOLD:
            gold_ps = ps.tile([C, C], F32, tag="gps", bufs=1, name="gold_ps")
            NGC = NCOL // C                            # 128 matmul chunks
            gpos = [0]
            gout = sb.tile([C, 2 * C], BF16, name="gout")

            def gold_trans():
                # cnt o T multiply on Pool (idle during the scan)
                nc.gpsimd.tensor_tensor(gout[:, C:2 * C], cnt_sb[:], tsb[:],
                                        op=OP.mult)

            def gold_mm(nmm, anchor=None):
                from concourse.tile_rust import add_dep_helper
                for i in range(nmm):
                    m = gpos[0]
                    if m >= NGC:
                        return
                    gpos[0] += 1
                    gi = nc.tensor.matmul(
                        gold_ps[:], hem[:, m * C:(m + 1) * C],
                        raw[:, m * C:(m + 1) * C],
                        start=(m == 0), stop=(m == NGC - 1))
                    if i == 0 and anchor is not None:
                        # pin the batch into this round's PE idle window --
                        # Tile otherwise hoists it between the scan matmuls
                        add_dep_helper(gi.ins, anchor.ins,
                                       reason="gold batch after scan matmul")

            def gold_finish(anchor=None):
                # ship the raw gold matrix: one idle-ACT copy out of PSUM,
                # no DVE work at all; host sums the diagonal in f64
                nc.scalar.copy(gout[:, 0:C], gold_ps[:])
                nc.gpsimd.dma_start(gold[:], gout[:])
        else:
            def gold_trans():
                pass

            def gold_mm(nmm, anchor=None):
                pass

            def gold_finish(anchor=None):
                zg = sb.tile([C, 2 * C], BF16, name="zg")
                nc.vector.memset(zg[:], 0.0)
                nc.sync.dma_start(gold[:], zg[:])

        if not _EN_SCAN:
            zr = sb.tile([1, W], F32, name="zr")
            nc.vector.memset(zr[:], 1.0)
            nc.sync.dma_start(cso_out[:], zr[:])
            nc.sync.dma_start(csf_out[:], zr[:])
            for c in range(len(CHUNKS)):
                exp_chunk(c)
            gold_trans()
            gold_mm(NCOL // C if _EN_GOLD else 0)
            gold_finish()
            nc.compile()
            return nc

        # exp the slices the first scan step needs, smallest-first so each
        # consumer (init matmuls, then the two step-1 mults) starts ASAP
        o1, n1 = CHUNKS[1]
        nc.scalar.activation(E[:, 0:BC], raw[:, 0:BC], AF.Exp, bias=bias[:])
        nc.scalar.activation(E[:, o1:o1 + G0W], raw[:, o1:o1 + G0W],
                             AF.Exp, bias=bias[:])
        nc.scalar.activation(E[:, o1 + G0W:o1 + n1], raw[:, o1 + G0W:o1 + n1],
                             AF.Exp, bias=bias[:])
        ones_mv = sb.tile([C, G1W], BF16, name="ones_mv")
        nc.vector.memset(ones_mv[:], 1.0)

        def extract_mm(state0, state1):
            # colsums via ones-matmul into PSUM (held there until copied)
            c0 = ps.tile([1, G0W], F32, tag="cs0", bufs=1, name="c0")
            c1 = ps.tile([1, G1W], F32, tag="cs1", bufs=1, name="c1")
            nc.tensor.matmul(c0[:], ones_col[:], state0, start=True, stop=True)
            nc.tensor.matmul(c1[:], ones_col[:], state1, start=True, stop=True)
            return c0, c1

        def extract_out(c0, c1, row_sb, row_out, copy_eng):
            if copy_eng == "act":
                nc.scalar.copy(row_sb[0:1, 0:G0W], c0[:])
                nc.scalar.copy(row_sb[0:1, G0W:W], c1[:])
            else:  # tail: run the two copies on parallel engines
                nc.vector.tensor_copy(row_sb[0:1, 0:G0W], c0[:])
                nc.scalar.copy(row_sb[0:1, G0W:W], c1[:])
            nc.sync.dma_start(row_out[:], row_sb[:])

        # ---- the scan ---------------------------------------------------
        # state_0 = E at each chain's local step 0, read in place
        st0 = None   # group tiles; step 1 reads E directly
        st1 = None
        mm1 = prev_mm1 = None
        for kk in range(1, L):
            pp0 = ps.tile([C, G0W], F32, tag="pp0", bufs=2, name=f"pp0_{kk}")
            pp1 = ps.tile([C, G1W], F32, tag="pp1", bufs=2, name=f"pp1_{kk}")
            if kk == 1:
                # chain 0 inits exactly from E_0; chains j>0 warm-start
                # from ones (only the direction must converge)
                nc.tensor.matmul(pp0[:, 0:BC], wf[:], E[:, 0:BC],
                                 start=True, stop=True)
                nc.tensor.matmul(pp0[:, BC:G0W], wf[:],
                                 ones_mv[:, 0:G0W - BC],
                                 start=True, stop=True)
                nc.tensor.matmul(pp1[:], wf[:], ones_mv[:],
                                 start=True, stop=True)
            else:
                nc.tensor.matmul(pp0[:], wf[:], st0, start=True, stop=True)
                prev_mm1 = mm1
                mm1 = nc.tensor.matmul(pp1[:], wf[:], st1,
                                       start=True, stop=True)

            a0 = wk.tile([C, G0W], BF16, tag="a0", bufs=3, name=f"a0_{kk}")
            a1 = wk.tile([C, G1W], BF16, tag="a1", bufs=3, name=f"a1_{kk}")
            o0, n0 = e_rng(kk, 0)
            o1, n1 = e_rng(kk, 1)
            mi0 = nc.vector.tensor_tensor(a0[:], pp0[:], E[:, o0:o0 + n0],
                                          op=OP.mult)
            nc.vector.tensor_tensor(a1[:], pp1[:], E[:, o1:o1 + n1],
                                    op=OP.mult)
            st0, st1 = a0[:], a1[:]

            if kk == O:
                cso_ps = extract_mm(st0, st1)
            if kk == 20:
                # the PSUM->SBUF copies run here, when ACT is done with exp
                extract_out(cso_ps[0], cso_ps[1], cso_sb, cso_out, "act")

            # stream exp 2+ rounds ahead of consumption
            if 1 <= kk <= 3:
                exp_chunk(kk + 1)
            elif kk == 4:
                exp_chunk(5)
                exp_chunk(6)
            elif kk == 5:
                exp_chunk(7)
            elif kk == 6:
                exp_chunk(8)
            elif kk == 10:
                # chunk 0's remainder (block DE): first re-read at round DE
                nc.scalar.activation(E[:, BC:W + BC], raw[:, BC:W + BC],
                                     AF.Exp, bias=bias[:])
                exp_chunk(11)
            elif kk in (8, 12, 14, 16):
                exp_chunk({8: 9, 12: 10, 14: 12, 16: 13}[kk])
            if kk == 14:
                gold_trans()
            # gold matmuls ride the idle PE slots once hemit has landed
            if kk >= 10:
                gold_mm(7, prev_mm1)

        gold_mm(NGC if _EN_GOLD else 0)   # any leftovers
        gold_finish(mi0)                  # ready before the final states
        cf = extract_mm(st0, st1)
        extract_out(cf[0], cf[1], csf_sb, csf_out, "split")

    nc.compile()
    return nc


# stored column -> (batch row, time) maps, shared by et and hemit prep
_COL_B = np.empty(NCOL, dtype=np.int64)
_COL_T = np.empty(NCOL, dtype=np.int64)
for _k in _ORDER:
    if _k <= O:
        _sl = slice(OFF[_k], OFF[_k] + BC)
        _COL_B[_sl] = np.arange(BC)
        _COL_T[_sl] = _k
    else:
        _sl = slice(OFF[_k], OFF[_k] + W)
        _COL_B[_sl] = np.tile(np.arange(BC), K)
        _COL_T[_sl] = np.repeat(np.arange(K) * DE + _k, BC)


def _prep_inputs(emissions, tags, mask, transitions):
    em = np.asarray(emissions, dtype=np.float32)
    tg = np.asarray(tags).astype(np.int64)
    mk = np.asarray(mask).astype(np.float32)
    tr = np.ascontiguousarray(np.asarray(transitions, dtype=np.float32))

    afwd = np.exp(tr.astype(np.float64)).astype(ml_dtypes.bfloat16)
    ident = np.eye(C, dtype=ml_dtypes.bfloat16)

    in_maps = []
    for core in range(NCORES):
        b0 = core * BC
        ec = em[b0:b0 + BC]                        # [BC,S,C]
        ett = ec.transpose(2, 1, 0)                # [C,S,BC]
        et = np.ascontiguousarray(
            ett[:, _COL_T, _COL_B]).astype(ml_dtypes.float8_e4m3fn)

        tgc = tg[b0:b0 + BC]
        mkc = mk[b0:b0 + BC]
        hemit = np.zeros((C, NCOL), dtype=ml_dtypes.float8_e4m3fn)
        hemit[tgc[_COL_B, _COL_T], np.arange(NCOL)] = \
            mkc[_COL_B, _COL_T].astype(ml_dtypes.float8_e4m3fn)

        cnt = np.zeros((C, C), dtype=np.float64)
        np.add.at(cnt, (tgc[:, :-1].ravel(), tgc[:, 1:].ravel()),
                  mkc[:, 1:].ravel().astype(np.float64))
        cnt = cnt.astype(np.float32)

        in_maps.append({
            "et": et, "hemit": hemit, "afwd": afwd,
            "cnt": cnt, "tsb": tr, "ident": ident,
        })
    return in_maps


def kernel(emissions, tags, mask, transitions, _trace=False):
    global _NC_CACHE
    if _NC_CACHE is None:
        _NC_CACHE = _build_nc()
    nc = _NC_CACHE

    in_maps = _prep_inputs(emissions, tags, mask, transitions)
    res = run_bass_kernel_spmd(
        nc, in_maps, core_ids=list(range(NCORES)), trace=_trace,
    )
    partition = np.float64(0.0)
    gold = np.float64(0.0)
    for r in res.results:
        n = np.asarray(r["csf"], dtype=np.float64).reshape(K, BC)
        g = np.asarray(r["cso"], dtype=np.float64).reshape(K, BC)
        logZ = np.log(n[K - 1]) + MU * S
        logZ += (np.log(n[:K - 1]) - np.log(g[1:])).sum(axis=0)
        partition += logZ.sum()
        gm = np.asarray(r["gold"], dtype=np.float64)
        gold += np.trace(gm[:, :C]) + gm[:, C:].sum()
    out = np.float32(partition - gold)
    if _trace:
        return out, res
    return out


# revision 61
# speedup vs baseline: 1.0180x; 1.0037x over previous
"""CRF negative-log-likelihood kernel for Trainium2 (8 NeuronCores).

Math: reference computes  partition - gold  where
  partition = sum_b logsumexp_c(alpha[511])  via the forward algorithm
  gold      = sum emissions[b,s,tags] * m + sum T[tags[s],tags[s+1]] * m[:,1:]

Device strategy (data-parallel over batch, 32 rows per core):
  * Linear domain: alpha_t = E_t o (A^T alpha_{t-1}) with A = exp(T),
    E_t = exp(e_t - MU).  One [128,128]x[128,W] matmul (PE) plus one
    elementwise multiply (DVE) per step.
  * K=23 overlapping forward chains cut the serial depth from 511 steps
    to L-1=27.  Chain j starts at t = j*DELTA from the raw emission
    vector E_{j*DELTA} and runs L=28 steps; its first O=5 steps are
    warm-up inside chain j-1's range.  Products of >=5 random positive
    matrices are numerically rank-1 (Perron-Frobenius contraction), so
    the chains glue exactly through two column-sum scalars per junction:
      logZ_b = log n[K-1] + sum_j (log n[j-1] - log gamma[j]) + MU*S
    where gamma[j] = colsum of chain j's state after its warm-up step O
    and n[j] = colsum at its final step (both measure t = j*DELTA + O
    resp. j*DELTA + L-1; the grid aligns junctions exactly).  Host takes
    the logs in f64.  Validated: junction error ~1e-16, total loss
    rel err ~3e-5 (bf16/fp8 rounding dominated).
  * No renormalisation: the exp bias -MU keeps per-step growth ~1, and
    a 28-step chain drifts far less than the f32/bf16 exponent range.
  * The scan runs as G=2 independent chain-groups (12+11 chains wide)
    round-robined so the DVE (the bottleneck engine: 125ns PSUM-access
    init + 1.04ns/col) stays saturated while semaphore round-trips hide.
  * Emissions arrive as fp8-e4m3 (halves DMA; validated noise ~1e-4) in
    a step-major block layout so DMA+exp stream strictly ahead of
    consumption and every scan-step read is one contiguous slice.
  * Gold emit: sum(raw o onehot(tags)) via PE: 128 PSUM-accumulated
    fp8 matmuls H_c^T R_c (diag trick), injected into scan-idle PE
    slots; diag extracted with an identity multiply + free-axis reduce.
  * Gold trans: host-built pair-count matrix CNT (index-only prep),
    mul+reduce against T on Pool/DVE.
  * Startup/tail: activation-table load and PE p-state ramp pre-warmed
    under the DMA shadow; weights on the parallel Pool DMA queue; gold
    reduction finishes on Pool so its DMA overlaps the csf output path.
Outputs per core: two colsum rows + gold column; host sums in float64.
"""

import sys

for _p in ("/opt/trn_rl_repo",):
    if _p not in sys.path:
        sys.path.insert(0, _p)

import os as _os
import numpy as np
import ml_dtypes
from contextlib import ExitStack

from concourse import bass, tile, mybir, bacc
from concourse.bass_utils import run_bass_kernel_spmd

NCORES = 8
B, S, C = 256, 512, 128
BC = B // NCORES          # batch rows per core
K = 23                    # chains
O = 5                     # warm-up steps per chain
DE = 22                   # chain start stride (DELTA)
L = DE + O + 1            # steps per chain (incl. init step 0)
MU = 5.85                 # exp prescale; host adds MU*S back per batch row
W = K * BC                # 736: full state width
G0W = 12 * BC             # group 0: chains 0..11  (384 cols)
G1W = 11 * BC             # group 1: chains 12..22 (352 cols)
NCOL = S * BC             # 16384 stored emission columns per core
assert K * DE == S - 1 - O and (K - 1) * DE + L - 1 == S - 1

# stored block order = consumption order: small block BLK(k) (chain 0's
# 32-col tile for t=k) directly before big block BLK(k+DE) for k=0..O,
# then big BLK(O+1..DE-1).  BLK(k>O) holds slot j = chain j's tile for
# t = j*DE + k (K*32 cols).  Small-before-big makes every warm-up read
# [chain0 | chains 1..11] one contiguous 384-col slice.
_ORDER = []
for _k in range(O + 1):
    _ORDER += [_k, DE + _k]
_ORDER += list(range(O + 1, DE))
OFF = {}
_pos = 0
for _k in _ORDER:
    OFF[_k] = _pos
    _pos += W if _k > O else BC
assert _pos == NCOL

# exp chunks: (offset, size) pairs in stored order.  Chunk c<=5 feeds
# scan step c (and steps 22..27); bulk chunk 6+i (2 blocks) feeds steps
# 6+2i and 7+2i.
CHUNKS = []
for _i in range(O + 1):                       # 6 chunks of 768
    CHUNKS.append((_i * (W + BC), W + BC))
_base = (O + 1) * (W + BC)
for _i in range(8):                           # 8 chunks of 2*736
    CHUNKS.append((_base + _i * 2 * W, 2 * W))
assert CHUNKS[-1][0] + CHUNKS[-1][1] == NCOL

# DMA batches (HWDGE costs 625ns per dma_start; batch the tail, but keep
# the early chunks separate so each lands — and unblocks its exp — ASAP).
# Chains j>0 warm-start from ones, so round 0 needs only the 32-col s0
# slice; block DE+0 (chunk 0's remainder) isn't read again until round
# DE, letting its DMA+exp drop out of the startup chain entirely.
_D1 = 3 * (W + BC)
DMA_BATCHES = [CHUNKS[1], (0, BC), CHUNKS[2],
               (_D1, _base - _D1), (_base, 2 * W),
               (_base + 2 * W, NCOL - _base - 2 * W), (BC, W)]

F32 = mybir.dt.float32
BF16 = mybir.dt.bfloat16
FP8 = mybir.dt.float8e4
AF = mybir.ActivationFunctionType
OP = mybir.AluOpType

_EN_GOLD = _os.environ.get("CRF_GOLD", "1") == "1"
_EN_SCAN = _os.environ.get("CRF_SCAN", "1") == "1"

_NC_CACHE = None


def _build_nc():
    nc = bacc.Bacc("TRN2", target_bir_lowering=False, debug=False)

    et_in = nc.dram_tensor("et", [C, NCOL], FP8, kind="ExternalInput").ap()
    hemit_in = nc.dram_tensor("hemit", [C, NCOL], FP8,
                              kind="ExternalInput").ap()
    afwd = nc.dram_tensor("afwd", [C, C], BF16, kind="ExternalInput").ap()
    cnt_in = nc.dram_tensor("cnt", [C, C], F32, kind="ExternalInput").ap()
    tsb_in = nc.dram_tensor("tsb", [C, C], F32, kind="ExternalInput").ap()
    cso_out = nc.dram_tensor("cso", [C, 6], F32, kind="ExternalOutput").ap()
    csf_out = nc.dram_tensor("csf", [C, 6], F32, kind="ExternalOutput").ap()
    gold = nc.dram_tensor("gold", [C, 2 * C], BF16,
                          kind="ExternalOutput").ap()

    with tile.TileContext(nc) as tc, ExitStack() as ctx:
        sb = ctx.enter_context(tc.tile_pool(name="sb", bufs=1))
        wk = ctx.enter_context(tc.tile_pool(name="wk", bufs=4))
        ps = ctx.enter_context(tc.tile_pool(name="ps", bufs=2, space="PSUM"))

        # ---- persistent tiles -------------------------------------------
        bias = sb.tile([C, 1], F32, name="bias")
        nc.vector.memset(bias[:], -MU)
        ones_col = sb.tile([C, 1], BF16, name="ones_col")
        nc.vector.memset(ones_col[:], 1.0)
        # dummy exp: pulls the 1283ns activation-table load into the DMA
        # shadow at t=0
        warm = sb.tile([C, 1], BF16, name="warm")
        nc.scalar.activation(warm[:], bias[:], AF.Exp, bias=bias[:])
        # dummy matmul: starts the PE p-state ramp clock (3us to full
        # speed) during the DMA shadow so round 0 runs at full clock
        pwarm = ps.tile([1, 1], F32, tag="pw", bufs=1, name="pwarm")
        nc.tensor.matmul(pwarm[:], ones_col[:], ones_col[:],
                         start=True, stop=True)

        raw = sb.tile([C, NCOL], FP8, name="raw")
        E = sb.tile([C, NCOL], BF16, name="E")
        hem = sb.tile([C, NCOL], FP8, name="hem")
        wf = sb.tile([C, C], BF16, name="wf")
        cso_sb = sb.tile([C, 6], F32, name="cso_sb")
        csf_sb = sb.tile([C, 6], F32, name="csf_sb")

        # ---- input DMA: et batches in consumption order (weights after
        # the early batches: not needed until the first matmul), then
        # hemit (needed mid-scan for gold), then the small gold inputs ----
        # weights ride the otherwise-idle Pool SWDGE queue, in parallel
        # with the emission stream on the SP queue
        nc.gpsimd.dma_start(wf[:], afwd[:])
        for o, n in DMA_BATCHES:
            nc.sync.dma_start(raw[:, o:o + n], et_in[:, o:o + n])
        hq = NCOL // 2
        for i in range(2):
            nc.sync.dma_start(hem[:, i * hq:(i + 1) * hq],
                              hemit_in[:, i * hq:(i + 1) * hq])
        cnt_sb = sb.tile([C, C], F32, name="cnt_sb")
        tsb = sb.tile([C, C], F32, name="tsb_t")
        nc.sync.dma_start(cnt_sb[:], cnt_in[:])
        nc.sync.dma_start(tsb[:], tsb_in[:])

        def exp_chunk(c, split=0):
            o, n = CHUNKS[c]
            if split:
                nc.scalar.activation(E[:, o:o + split], raw[:, o:o + split],
                                     AF.Exp, bias=bias[:])
                nc.scalar.activation(E[:, o + split:o + n],
                                     raw[:, o + split:o + n],
                                     AF.Exp, bias=bias[:])
            else:
                nc.scalar.activation(E[:, o:o + n], raw[:, o:o + n], AF.Exp,
                                     bias=bias[:])

        # E source slice for (step, group).  Small-before-big block order
        # makes group 0's warm-up read [chain0 | big-block slots 0..10]
        # contiguous, so both groups always read one slice starting at
        # OFF[kk] (the small block for kk<=O, the big block otherwise).
        def e_rng(kk, grp):
            if grp == 0:
                return OFF[kk], G0W
            return OFF[kk] + G0W, G1W

        # gold state
        if _EN_GOLD:
            gold_ps = ps.tile([C, C], F32, tag="gps", bufs=1, name="gold_ps")
            NGC = NCOL // C                            # 128 matmul chunks
            gpos = [0]
            gout = sb.tile([C, 2 * C], BF16, name="gout")

            def gold_trans():
                # cnt o T multiply on Pool (idle during the scan)
                nc.gpsimd.tensor_tensor(gout[:, C:2 * C], cnt_sb[:], tsb[:],
                                        op=OP.mult)

            def gold_mm(nmm, anchor=None):
                from concourse.tile_rust import add_dep_helper
                for i in range(nmm):
                    m = gpos[0]
                    if m >= NGC:
                        return
                    gpos[0] += 1
                    gi = nc.tensor.matmul(
                        gold_ps[:], hem[:, m * C:(m + 1) * C],
                        raw[:, m * C:(m + 1) * C],
                        start=(m == 0), stop=(m == NGC - 1))
                    if i == 0 and anchor is not None:
                        # pin the batch into this round's PE idle window --
                        # Tile otherwise hoists it between the scan matmuls
                        add_dep_helper(gi.ins, anchor.ins,
                                       reason="gold batch after scan matmul")

            def gold_finish(anchor=None):
                # ship the raw gold matrix: one idle-ACT copy out of PSUM,
                # no DVE work at all; host sums the diagonal in f64
                nc.scalar.copy(gout[:, 0:C], gold_ps[:])
                nc.gpsimd.dma_start(gold[:], gout[:])
        else:
            def gold_trans():
                pass

            def gold_mm(nmm, anchor=None):
                pass

            def gold_finish(anchor=None):
                zg = sb.tile([C, 2 * C], BF16, name="zg")
                nc.vector.memset(zg[:], 0.0)
                nc.sync.dma_start(gold[:], zg[:])

        if not _EN_SCAN:
            zr = sb.tile([C, 6], F32, name="zr")
            nc.vector.memset(zr[:], 1.0)
            nc.sync.dma_start(cso_out[:], zr[:])
            nc.sync.dma_start(csf_out[:], zr[:])
            for c in range(len(CHUNKS)):
                exp_chunk(c)
            gold_trans()
            gold_mm(NCOL // C if _EN_GOLD else 0)
            gold_finish()
            nc.compile()
            return nc

        # exp the slices the first scan step needs, smallest-first so each
        # consumer (init matmuls, then the two step-1 mults) starts ASAP
        o1, n1 = CHUNKS[1]
        nc.scalar.activation(E[:, 0:BC], raw[:, 0:BC], AF.Exp, bias=bias[:])
        nc.scalar.activation(E[:, o1:o1 + G0W], raw[:, o1:o1 + G0W],
                             AF.Exp, bias=bias[:])
        nc.scalar.activation(E[:, o1 + G0W:o1 + n1], raw[:, o1 + G0W:o1 + n1],
                             AF.Exp, bias=bias[:])
        ones_mv = sb.tile([C, G1W], BF16, name="ones_mv")
        nc.vector.memset(ones_mv[:], 1.0)

        colt = ps.tile([C, 12], F32, tag="cst", bufs=1, name="colt")
        nc.vector.memset(colt[:], 1.0)

        def extract_mm(t0, t1, base):
            # colsums land in the PARTITION dim: state chunk as stationary,
            # ones as moving -> out[p, 0] = colsum of state column p.  The
            # later PSUM->SBUF copy is then [C, 6] (6 free elements) instead
            # of a 736-element row.
            for i, (t, a, wdt) in enumerate(
                    [(t0, 0, C), (t0, C, C), (t0, 2 * C, C),
                     (t1, 0, C), (t1, C, C), (t1, 2 * C, G1W - 2 * C)]):
                nc.tensor.matmul(colt[0:wdt, base + i:base + i + 1],
                                 t[:, a:a + wdt], ones_col[:],
                                 start=True, stop=True)

        def extract_out(base, row_sb, row_out, copy_eng):
            if copy_eng == "act":
                nc.scalar.copy(row_sb[:], colt[:, base:base + 6])
            else:
                nc.vector.tensor_copy(row_sb[:], colt[:, base:base + 6])
            nc.sync.dma_start(row_out[:], row_sb[:])

        # ---- the scan ---------------------------------------------------
        # state_0 = E at each chain's local step 0, read in place
        st0 = None   # group tiles; step 1 reads E directly
        st1 = None
        mm1 = prev_mm1 = None
        for kk in range(1, L):
            pp0 = ps.tile([C, G0W], F32, tag="pp0", bufs=2, name=f"pp0_{kk}")
            pp1 = ps.tile([C, G1W], F32, tag="pp1", bufs=2, name=f"pp1_{kk}")
            if kk == 1:
                # chain 0 inits exactly from E_0; chains j>0 warm-start
                # from ones (only the direction must converge)
                nc.tensor.matmul(pp0[:, 0:BC], wf[:], E[:, 0:BC],
                                 start=True, stop=True)
                nc.tensor.matmul(pp0[:, BC:G0W], wf[:],
                                 ones_mv[:, 0:G0W - BC],
                                 start=True, stop=True)
                nc.tensor.matmul(pp1[:], wf[:], ones_mv[:],
                                 start=True, stop=True)
            else:
                nc.tensor.matmul(pp0[:], wf[:], st0, start=True, stop=True)
                prev_mm1 = mm1
                mm1 = nc.tensor.matmul(pp1[:], wf[:], st1,
                                       start=True, stop=True)

            a0 = wk.tile([C, G0W], BF16, tag="a0", bufs=3, name=f"a0_{kk}")
            a1 = wk.tile([C, G1W], BF16, tag="a1", bufs=3, name=f"a1_{kk}")
            o0, n0 = e_rng(kk, 0)
            o1, n1 = e_rng(kk, 1)
            mi0 = nc.vector.tensor_tensor(a0[:], pp0[:], E[:, o0:o0 + n0],
                                          op=OP.mult)
            nc.vector.tensor_tensor(a1[:], pp1[:], E[:, o1:o1 + n1],
                                    op=OP.mult)
            st0, st1 = a0[:], a1[:]

            if kk == O:
                extract_mm(a0, a1, 0)
            if kk == 20:
                # the PSUM->SBUF copy runs here, when ACT is done with exp
                extract_out(0, cso_sb, cso_out, "act")

            # stream exp 2+ rounds ahead of consumption
            if 1 <= kk <= 3:
                exp_chunk(kk + 1)
            elif kk == 4:
                exp_chunk(5)
                exp_chunk(6)
            elif kk == 5:
                exp_chunk(7)
            elif kk == 6:
                exp_chunk(8)
            elif kk == 10:
                # chunk 0's remainder (block DE): first re-read at round DE
                nc.scalar.activation(E[:, BC:W + BC], raw[:, BC:W + BC],
                                     AF.Exp, bias=bias[:])
                exp_chunk(11)
            elif kk in (8, 12, 14, 16):
                exp_chunk({8: 9, 12: 10, 14: 12, 16: 13}[kk])
            if kk == 14:
                gold_trans()
            # gold matmuls ride the idle PE slots once hemit has landed
            if kk >= 10:
                gold_mm(7, prev_mm1)

        gold_mm(NGC if _EN_GOLD else 0)   # any leftovers
        gold_finish(mi0)                  # ready before the final states
        extract_mm(a0, a1, 6)
        extract_out(6, csf_sb, csf_out, "dve")

    nc.compile()
    return nc


# stored column -> (batch row, time) maps, shared by et and hemit prep
_COL_B = np.empty(NCOL, dtype=np.int64)
_COL_T = np.empty(NCOL, dtype=np.int64)
for _k in _ORDER:
    if _k <= O:
        _sl = slice(OFF[_k], OFF[_k] + BC)
        _COL_B[_sl] = np.arange(BC)
        _COL_T[_sl] = _k
    else:
        _sl = slice(OFF[_k], OFF[_k] + W)
        _COL_B[_sl] = np.tile(np.arange(BC), K)
        _COL_T[_sl] = np.repeat(np.arange(K) * DE + _k, BC)


def _prep_inputs(emissions, tags, mask, transitions):
    em = np.asarray(emissions, dtype=np.float32)
    tg = np.asarray(tags).astype(np.int64)
    mk = np.asarray(mask).astype(np.float32)
    tr = np.ascontiguousarray(np.asarray(transitions, dtype=np.float32))

    afwd = np.exp(tr.astype(np.float64)).astype(ml_dtypes.bfloat16)

    in_maps = []
    for core in range(NCORES):
        b0 = core * BC
        ec = em[b0:b0 + BC]                        # [BC,S,C]
        ett = ec.transpose(2, 1, 0)                # [C,S,BC]
        et = np.ascontiguousarray(
            ett[:, _COL_T, _COL_B]).astype(ml_dtypes.float8_e4m3fn)

        tgc = tg[b0:b0 + BC]
        mkc = mk[b0:b0 + BC]
        hemit = np.zeros((C, NCOL), dtype=ml_dtypes.float8_e4m3fn)
        hemit[tgc[_COL_B, _COL_T], np.arange(NCOL)] = \
            mkc[_COL_B, _COL_T].astype(ml_dtypes.float8_e4m3fn)

        cnt = np.zeros((C, C), dtype=np.float64)
        np.add.at(cnt, (tgc[:, :-1].ravel(), tgc[:, 1:].ravel()),
                  mkc[:, 1:].ravel().astype(np.float64))
        cnt = cnt.astype(np.float32)

        in_maps.append({
            "et": et, "hemit": hemit, "afwd": afwd,
            "cnt": cnt, "tsb": tr,
        })
    return in_maps


def kernel(emissions, tags, mask, transitions, _trace=False):
    global _NC_CACHE
    if _NC_CACHE is None:
        _NC_CACHE = _build_nc()
    nc = _NC_CACHE

    in_maps = _prep_inputs(emissions, tags, mask, transitions)
    res = run_bass_kernel_spmd(
        nc, in_maps, core_ids=list(range(NCORES)), trace=_trace,
    )
    partition = np.float64(0.0)
    gold = np.float64(0.0)
    cmap = np.empty(W, dtype=np.int64)
    for i in range(6):
        a = i * C if i < 3 else G0W + (i - 3) * C
        wdt = min(C, W - a)
        cmap[a:a + wdt] = np.arange(wdt) * 6 + i
    for r in res.results:
        n = np.asarray(r["csf"], dtype=np.float64).ravel()[cmap].reshape(K, BC)
        g = np.asarray(r["cso"], dtype=np.float64).ravel()[cmap].reshape(K, BC)
        logZ = np.log(n[K - 1]) + MU * S
        logZ += (np.log(n[:K - 1]) - np.log(g[1:])).sum(axis=0)
        partition += logZ.sum()
        gm = np.asarray(r["gold"], dtype=np.float64)
        gold += np.trace(gm[:, :C]) + gm[:, C:].sum()
    out = np.float32(partition - gold)
    if _trace:
        return out, res
    return out


# revision 62
# speedup vs baseline: 1.0397x; 1.0213x over previous
"""CRF negative-log-likelihood kernel for Trainium2 (8 NeuronCores).

Math: reference computes  partition - gold  where
  partition = sum_b logsumexp_c(alpha[511])  via the forward algorithm
  gold      = sum emissions[b,s,tags] * m + sum T[tags[s],tags[s+1]] * m[:,1:]

Device strategy (data-parallel over batch, 32 rows per core):
  * Linear domain: alpha_t = E_t o (A^T alpha_{t-1}) with A = exp(T),
    E_t = exp(e_t - MU).  One [128,128]x[128,W] matmul (PE) plus one
    elementwise multiply (DVE) per step.
  * K=23 overlapping forward chains cut the serial depth from 511 steps
    to L-1=27.  Chain j starts at t = j*DELTA from the raw emission
    vector E_{j*DELTA} and runs L=28 steps; its first O=5 steps are
    warm-up inside chain j-1's range.  Products of >=5 random positive
    matrices are numerically rank-1 (Perron-Frobenius contraction), so
    the chains glue exactly through two column-sum scalars per junction:
      logZ_b = log n[K-1] + sum_j (log n[j-1] - log gamma[j]) + MU*S
    where gamma[j] = colsum of chain j's state after its warm-up step O
    and n[j] = colsum at its final step (both measure t = j*DELTA + O
    resp. j*DELTA + L-1; the grid aligns junctions exactly).  Host takes
    the logs in f64.  Validated: junction error ~1e-16, total loss
    rel err ~3e-5 (bf16/fp8 rounding dominated).
  * No renormalisation: the exp bias -MU keeps per-step growth ~1, and
    a 28-step chain drifts far less than the f32/bf16 exponent range.
  * The scan runs as G=2 independent chain-groups (12+11 chains wide)
    round-robined so the DVE (the bottleneck engine: 125ns PSUM-access
    init + 1.04ns/col) stays saturated while semaphore round-trips hide.
  * Emissions arrive as fp8-e4m3 (halves DMA; validated noise ~1e-4) in
    a step-major block layout so DMA+exp stream strictly ahead of
    consumption and every scan-step read is one contiguous slice.
  * Gold emit: sum(raw o onehot(tags)) via PE: 128 PSUM-accumulated
    fp8 matmuls H_c^T R_c (diag trick), injected into scan-idle PE
    slots; diag extracted with an identity multiply + free-axis reduce.
  * Gold trans: host-built pair-count matrix CNT (index-only prep),
    mul+reduce against T on Pool/DVE.
  * Startup/tail: activation-table load and PE p-state ramp pre-warmed
    under the DMA shadow; weights on the parallel Pool DMA queue; gold
    reduction finishes on Pool so its DMA overlaps the csf output path.
Outputs per core: two colsum rows + gold column; host sums in float64.
"""

import sys

for _p in ("/opt/trn_rl_repo",):
    if _p not in sys.path:
        sys.path.insert(0, _p)

import os as _os
import numpy as np
import ml_dtypes
from contextlib import ExitStack

from concourse import bass, tile, mybir, bacc
from concourse.bass_utils import run_bass_kernel_spmd

NCORES = 8
B, S, C = 256, 512, 128
BC = B // NCORES          # batch rows per core
K = 23                    # chains
O = 5                     # warm-up steps per chain
DE = 22                   # chain start stride (DELTA)
L = DE + O + 1            # steps per chain (incl. init step 0)
MU = 5.85                 # exp prescale; host adds MU*S back per batch row
W = K * BC                # 736: full state width
G0W = 12 * BC             # group 0: chains 0..11  (384 cols)
G1W = 11 * BC             # group 1: chains 12..22 (352 cols)
NCOL = S * BC             # 16384 stored emission columns per core
assert K * DE == S - 1 - O and (K - 1) * DE + L - 1 == S - 1

# stored block order = consumption order: small block BLK(k) (chain 0's
# 32-col tile for t=k) directly before big block BLK(k+DE) for k=0..O,
# then big BLK(O+1..DE-1).  BLK(k>O) holds slot j = chain j's tile for
# t = j*DE + k (K*32 cols).  Small-before-big makes every warm-up read
# [chain0 | chains 1..11] one contiguous 384-col slice.
_ORDER = [DE, 0]
for _k in range(1, O + 1):
    _ORDER += [_k, DE + _k]
_ORDER += list(range(O + 1, DE))
OFF = {}
_pos = 0
for _k in _ORDER:
    OFF[_k] = _pos
    _pos += W if _k > O else BC
assert _pos == NCOL

# exp chunks: (offset, size) pairs in stored order.  Chunk c<=5 feeds
# scan step c (and steps 22..27); bulk chunk 6+i (2 blocks) feeds steps
# 6+2i and 7+2i.
CHUNKS = [(0, W), (W, BC + BC + W)]           # B22 (late); s0+s1+B23
for _i in range(2, O + 1):                    # 4 chunks of 768
    CHUNKS.append((_i * (W + BC), W + BC))
_base = (O + 1) * (W + BC)
for _i in range(8):                           # 8 chunks of 2*736
    CHUNKS.append((_base + _i * 2 * W, 2 * W))
assert CHUNKS[-1][0] + CHUNKS[-1][1] == NCOL

# DMA batches (HWDGE costs 625ns per dma_start; batch the tail, but keep
# the early chunks separate so each lands — and unblocks its exp — ASAP).
# Chains j>0 warm-start from ones, so round 0 needs only the 32-col s0
# slice; block DE+0 (chunk 0's remainder) isn't read again until round
# DE, letting its DMA+exp drop out of the startup chain entirely.
_D1 = 3 * (W + BC)
DMA_BATCHES = [CHUNKS[1], CHUNKS[2],
               (_D1, _base - _D1), (_base, 2 * W),
               (_base + 2 * W, NCOL - _base - 2 * W), (0, W)]

F32 = mybir.dt.float32
BF16 = mybir.dt.bfloat16
FP8 = mybir.dt.float8e4
AF = mybir.ActivationFunctionType
OP = mybir.AluOpType

_EN_GOLD = _os.environ.get("CRF_GOLD", "1") == "1"
_EN_SCAN = _os.environ.get("CRF_SCAN", "1") == "1"

_NC_CACHE = None


def _build_nc():
    nc = bacc.Bacc("TRN2", target_bir_lowering=False, debug=False)

    et_in = nc.dram_tensor("et", [C, NCOL], FP8, kind="ExternalInput").ap()
    hemit_in = nc.dram_tensor("hemit", [C, NCOL], FP8,
                              kind="ExternalInput").ap()
    afwd = nc.dram_tensor("afwd", [C, C], BF16, kind="ExternalInput").ap()
    cnt_in = nc.dram_tensor("cnt", [C, C], F32, kind="ExternalInput").ap()
    tsb_in = nc.dram_tensor("tsb", [C, C], F32, kind="ExternalInput").ap()
    cso_out = nc.dram_tensor("cso", [C, 6], F32, kind="ExternalOutput").ap()
    csf_out = nc.dram_tensor("csf", [C, 6], F32, kind="ExternalOutput").ap()
    gold = nc.dram_tensor("gold", [C, 2 * C], BF16,
                          kind="ExternalOutput").ap()

    with tile.TileContext(nc) as tc, ExitStack() as ctx:
        sb = ctx.enter_context(tc.tile_pool(name="sb", bufs=1))
        wk = ctx.enter_context(tc.tile_pool(name="wk", bufs=4))
        ps = ctx.enter_context(tc.tile_pool(name="ps", bufs=2, space="PSUM"))

        # ---- persistent tiles -------------------------------------------
        bias = sb.tile([C, 1], F32, name="bias")
        nc.vector.memset(bias[:], -MU)
        ones_col = sb.tile([C, 1], BF16, name="ones_col")
        nc.vector.memset(ones_col[:], 1.0)
        # dummy exp: pulls the 1283ns activation-table load into the DMA
        # shadow at t=0
        warm = sb.tile([C, 1], BF16, name="warm")
        nc.scalar.activation(warm[:], bias[:], AF.Exp, bias=bias[:])
        # dummy matmul: starts the PE p-state ramp clock (3us to full
        # speed) during the DMA shadow so round 0 runs at full clock
        pwarm = ps.tile([1, 1], F32, tag="pw", bufs=1, name="pwarm")
        nc.tensor.matmul(pwarm[:], ones_col[:], ones_col[:],
                         start=True, stop=True)

        raw = sb.tile([C, NCOL], FP8, name="raw")
        E = sb.tile([C, NCOL], BF16, name="E")
        hem = sb.tile([C, NCOL], FP8, name="hem")
        wf = sb.tile([C, C], BF16, name="wf")
        cso_sb = sb.tile([C, 6], F32, name="cso_sb")
        csf_sb = sb.tile([C, 6], F32, name="csf_sb")

        # ---- input DMA: et batches in consumption order (weights after
        # the early batches: not needed until the first matmul), then
        # hemit (needed mid-scan for gold), then the small gold inputs ----
        # weights ride the otherwise-idle Pool SWDGE queue, in parallel
        # with the emission stream on the SP queue
        nc.gpsimd.dma_start(wf[:], afwd[:])
        for o, n in DMA_BATCHES:
            nc.sync.dma_start(raw[:, o:o + n], et_in[:, o:o + n])
        hq = NCOL // 2
        for i in range(2):
            nc.sync.dma_start(hem[:, i * hq:(i + 1) * hq],
                              hemit_in[:, i * hq:(i + 1) * hq])
        cnt_sb = sb.tile([C, C], F32, name="cnt_sb")
        tsb = sb.tile([C, C], F32, name="tsb_t")
        nc.sync.dma_start(cnt_sb[:], cnt_in[:])
        nc.sync.dma_start(tsb[:], tsb_in[:])

        def exp_chunk(c, split=0):
            o, n = CHUNKS[c]
            if split:
                nc.scalar.activation(E[:, o:o + split], raw[:, o:o + split],
                                     AF.Exp, bias=bias[:])
                nc.scalar.activation(E[:, o + split:o + n],
                                     raw[:, o + split:o + n],
                                     AF.Exp, bias=bias[:])
            else:
                nc.scalar.activation(E[:, o:o + n], raw[:, o:o + n], AF.Exp,
                                     bias=bias[:])

        # E source slice for (step, group).  Small-before-big block order
        # makes group 0's warm-up read [chain0 | big-block slots 0..10]
        # contiguous, so both groups always read one slice starting at
        # OFF[kk] (the small block for kk<=O, the big block otherwise).
        def e_rng(kk, grp):
            if grp == 0:
                return OFF[kk], G0W
            return OFF[kk] + G0W, G1W

        # gold state
        if _EN_GOLD:
            gold_ps = ps.tile([C, C], F32, tag="gps", bufs=1, name="gold_ps")
            NGC = NCOL // C                            # 128 matmul chunks
            gpos = [0]
            gout = sb.tile([C, 2 * C], BF16, name="gout")

            def gold_trans():
                # cnt o T multiply on Pool (idle during the scan)
                nc.gpsimd.tensor_tensor(gout[:, C:2 * C], cnt_sb[:], tsb[:],
                                        op=OP.mult)

            def gold_mm(nmm, anchor=None):
                from concourse.tile_rust import add_dep_helper
                for i in range(nmm):
                    m = gpos[0]
                    if m >= NGC:
                        return
                    gpos[0] += 1
                    gi = nc.tensor.matmul(
                        gold_ps[:], hem[:, m * C:(m + 1) * C],
                        raw[:, m * C:(m + 1) * C],
                        start=(m == 0), stop=(m == NGC - 1))
                    if i == 0 and anchor is not None:
                        # pin the batch into this round's PE idle window --
                        # Tile otherwise hoists it between the scan matmuls
                        add_dep_helper(gi.ins, anchor.ins,
                                       reason="gold batch after scan matmul")

            def gold_finish(anchor=None):
                # ship the raw gold matrix: one idle-ACT copy out of PSUM,
                # no DVE work at all; host sums the diagonal in f64
                nc.scalar.copy(gout[:, 0:C], gold_ps[:])
                nc.gpsimd.dma_start(gold[:], gout[:])
        else:
            def gold_trans():
                pass

            def gold_mm(nmm, anchor=None):
                pass

            def gold_finish(anchor=None):
                zg = sb.tile([C, 2 * C], BF16, name="zg")
                nc.vector.memset(zg[:], 0.0)
                nc.sync.dma_start(gold[:], zg[:])

        if not _EN_SCAN:
            zr = sb.tile([C, 6], F32, name="zr")
            nc.vector.memset(zr[:], 1.0)
            nc.sync.dma_start(cso_out[:], zr[:])
            nc.sync.dma_start(csf_out[:], zr[:])
            for c in range(len(CHUNKS)):
                exp_chunk(c)
            gold_trans()
            gold_mm(NCOL // C if _EN_GOLD else 0)
            gold_finish()
            nc.compile()
            return nc

        # exp the slices the first scan step needs: [s0|s1|B23 slots 0-10]
        # is one contiguous op covering chain-0's init and group 0's step-1
        # read; group 1's step-1 read follows
        nc.scalar.activation(E[:, W:W + BC + G0W], raw[:, W:W + BC + G0W],
                             AF.Exp, bias=bias[:])
        nc.scalar.activation(E[:, W + BC + G0W:2 * W + 2 * BC],
                             raw[:, W + BC + G0W:2 * W + 2 * BC],
                             AF.Exp, bias=bias[:])
        ones_mv = sb.tile([C, G1W], BF16, name="ones_mv")
        nc.vector.memset(ones_mv[:], 1.0)

        colt = ps.tile([C, 12], F32, tag="cst", bufs=1, name="colt")
        nc.vector.memset(colt[:], 1.0)

        def extract_mm(t0, t1, base):
            # colsums land in the PARTITION dim: state chunk as stationary,
            # ones as moving -> out[p, 0] = colsum of state column p.  The
            # later PSUM->SBUF copy is then [C, 6] (6 free elements) instead
            # of a 736-element row.
            for i, (t, a, wdt) in enumerate(
                    [(t0, 0, C), (t0, C, C), (t0, 2 * C, C),
                     (t1, 0, C), (t1, C, C), (t1, 2 * C, G1W - 2 * C)]):
                nc.tensor.matmul(colt[0:wdt, base + i:base + i + 1],
                                 t[:, a:a + wdt], ones_col[:],
                                 start=True, stop=True)

        def extract_out(base, row_sb, row_out, copy_eng):
            if copy_eng == "act":
                nc.scalar.copy(row_sb[:], colt[:, base:base + 6])
            else:
                nc.vector.tensor_copy(row_sb[:], colt[:, base:base + 6])
            nc.sync.dma_start(row_out[:], row_sb[:])

        # ---- the scan ---------------------------------------------------
        # state_0 = E at each chain's local step 0, read in place
        st0 = None   # group tiles; step 1 reads E directly
        st1 = None
        mm1 = prev_mm1 = None
        for kk in range(1, L):
            pp0 = ps.tile([C, G0W], F32, tag="pp0", bufs=2, name=f"pp0_{kk}")
            pp1 = ps.tile([C, G1W], F32, tag="pp1", bufs=2, name=f"pp1_{kk}")
            if kk == 1:
                # chain 0 inits exactly from E_0; chains j>0 warm-start
                # from ones (only the direction must converge)
                nc.tensor.matmul(pp0[:, 0:BC], wf[:], E[:, W:W + BC],
                                 start=True, stop=True)
                nc.tensor.matmul(pp0[:, BC:G0W], wf[:],
                                 ones_mv[:, 0:G0W - BC],
                                 start=True, stop=True)
                nc.tensor.matmul(pp1[:], wf[:], ones_mv[:],
                                 start=True, stop=True)
            else:
                nc.tensor.matmul(pp0[:], wf[:], st0, start=True, stop=True)
                prev_mm1 = mm1
                mm1 = nc.tensor.matmul(pp1[:], wf[:], st1,
                                       start=True, stop=True)

            a0 = wk.tile([C, G0W], BF16, tag="a0", bufs=3, name=f"a0_{kk}")
            a1 = wk.tile([C, G1W], BF16, tag="a1", bufs=3, name=f"a1_{kk}")
            o0, n0 = e_rng(kk, 0)
            o1, n1 = e_rng(kk, 1)
            mi0 = nc.vector.tensor_tensor(a0[:], pp0[:], E[:, o0:o0 + n0],
                                          op=OP.mult)
            nc.vector.tensor_tensor(a1[:], pp1[:], E[:, o1:o1 + n1],
                                    op=OP.mult)
            st0, st1 = a0[:], a1[:]

            if kk == O:
                extract_mm(a0, a1, 0)
            if kk == 20:
                # the PSUM->SBUF copy runs here, when ACT is done with exp
                extract_out(0, cso_sb, cso_out, "act")

            # stream exp 2+ rounds ahead of consumption
            if 1 <= kk <= 3:
                exp_chunk(kk + 1)
            elif kk == 4:
                exp_chunk(5)
                exp_chunk(6)
            elif kk == 5:
                exp_chunk(7)
            elif kk == 6:
                exp_chunk(8)
            elif kk == 10:
                exp_chunk(0)   # block DE: not read until round DE
                exp_chunk(11)
            elif kk in (8, 12, 14, 16):
                exp_chunk({8: 9, 12: 10, 14: 12, 16: 13}[kk])
            if kk == 14:
                gold_trans()
            # gold matmuls ride the idle PE slots once hemit has landed
            if kk >= 10:
                gold_mm(7, prev_mm1)

        gold_mm(NGC if _EN_GOLD else 0)   # any leftovers
        gold_finish(mi0)                  # ready before the final states
        extract_mm(a0, a1, 6)
        extract_out(6, csf_sb, csf_out, "dve")

    nc.compile()
    return nc


# stored column -> (batch row, time) maps, shared by et and hemit prep
_COL_B = np.empty(NCOL, dtype=np.int64)
_COL_T = np.empty(NCOL, dtype=np.int64)
for _k in _ORDER:
    if _k <= O:
        _sl = slice(OFF[_k], OFF[_k] + BC)
        _COL_B[_sl] = np.arange(BC)
        _COL_T[_sl] = _k
    else:
        _sl = slice(OFF[_k], OFF[_k] + W)
        _COL_B[_sl] = np.tile(np.arange(BC), K)
        _COL_T[_sl] = np.repeat(np.arange(K) * DE + _k, BC)


def _prep_inputs(emissions, tags, mask, transitions):
    em = np.asarray(emissions, dtype=np.float32)
    tg = np.asarray(tags).astype(np.int64)
    mk = np.asarray(mask).astype(np.float32)
    tr = np.ascontiguousarray(np.asarray(transitions, dtype=np.float32))

    afwd = np.exp(tr.astype(np.float64)).astype(ml_dtypes.bfloat16)

    in_maps = []
    for core in range(NCORES):
        b0 = core * BC
        ec = em[b0:b0 + BC]                        # [BC,S,C]
        ett = ec.transpose(2, 1, 0)                # [C,S,BC]
        et = np.ascontiguousarray(
            ett[:, _COL_T, _COL_B]).astype(ml_dtypes.float8_e4m3fn)

        tgc = tg[b0:b0 + BC]
        mkc = mk[b0:b0 + BC]
        hemit = np.zeros((C, NCOL), dtype=ml_dtypes.float8_e4m3fn)
        hemit[tgc[_COL_B, _COL_T], np.arange(NCOL)] = \
            mkc[_COL_B, _COL_T].astype(ml_dtypes.float8_e4m3fn)

        cnt = np.zeros((C, C), dtype=np.float64)
        np.add.at(cnt, (tgc[:, :-1].ravel(), tgc[:, 1:].ravel()),
                  mkc[:, 1:].ravel().astype(np.float64))
        cnt = cnt.astype(np.float32)

        in_maps.append({
            "et": et, "hemit": hemit, "afwd": afwd,
            "cnt": cnt, "tsb": tr,
        })
    return in_maps


def kernel(emissions, tags, mask, transitions, _trace=False):
    global _NC_CACHE
    if _NC_CACHE is None:
        _NC_CACHE = _build_nc()
    nc = _NC_CACHE

    in_maps = _prep_inputs(emissions, tags, mask, transitions)
    res = run_bass_kernel_spmd(
        nc, in_maps, core_ids=list(range(NCORES)), trace=_trace,
    )
    partition = np.float64(0.0)
    gold = np.float64(0.0)
    cmap = np.empty(W, dtype=np.int64)
    for i in range(6):
        a = i * C if i < 3 else G0W + (i - 3) * C
        wdt = min(C, W - a)
        cmap[a:a + wdt] = np.arange(wdt) * 6 + i
    for r in res.results:
        n = np.asarray(r["csf"], dtype=np.float64).ravel()[cmap].reshape(K, BC)
        g = np.asarray(r["cso"], dtype=np.float64).ravel()[cmap].reshape(K, BC)
        logZ = np.log(n[K - 1]) + MU * S
        logZ += (np.log(n[:K - 1]) - np.log(g[1:])).sum(axis=0)
        partition += logZ.sum()
        gm = np.asarray(r["gold"], dtype=np.float64)
        gold += np.trace(gm[:, :C]) + gm[:, C:].sum()
    out = np.float32(partition - gold)
    if _trace:
        return out, res
    return out
